# revision 26
# baseline (speedup 1.0000x reference)
"""Multi-head attention (B=2, S=2048, D=1024, H=16) on 8 Trainium2 NeuronCores.

Sharding: core c -> (batch b = c//4, head-group g = c%4).  Each core computes
Q/K/V projections for its 4 heads (256 features), causal attention for those
heads over the full sequence, and a partial O-projection (its 256 attn
features x full Wo.T slice).  The host sums the 4 bf16 partial outputs per
batch and folds in the biases that commute with the reduction (bo, bv @ Wo.T).

Pipeline (per core): the sequence is processed in 4 chunks of 512 tokens.
Stage c projects K/Q/V for chunk c, then runs causal attention for query
block c (which only needs K/V chunks 0..c), then the partial O-projection
for that block.  This starts the ScalarE exp stream (the serial bottleneck)
~45us earlier than projecting everything upfront.

Engine assignment:
  TensorE   projections + QK^T + PV + O-proj.  QK^T contracts over dk=64, so
            the two heads of a feature tile (partitions 0-63 / 64-127) issue
            back-to-back and run concurrently in different PE row groups
            (tile_position auto-derived from base partitions).
  ScalarE   only exp (fused 1/sqrt(dk) scale), one ACTIVATE per k-tile
            covering both heads of the pair via a [128, 2, QB] PSUM tile.
  VectorE   PSUM evacuations (+ K/Q bias add), mask multiplies, reciprocal,
            normalize multiplies.
  GpSimd    partition-broadcast of 1/denom; output DMA (SWDGE) so the Sync
            queue stays dedicated to input streaming.
  V layout  [tok, feat+1] with a ones column: PV also accumulates the
            softmax denominator (scores are O(5): no max subtraction needed,
            f32 exp cannot overflow).
"""

import hashlib
from contextlib import ExitStack

import ml_dtypes
import numpy as np

import concourse.bass as bass
import concourse.tile as tile
from concourse import bacc, hw_specs, mybir
from concourse.bass_utils import run_bass_kernel_spmd

# Calibrate the Tile scheduler's cost model to measured slice durations
# (ACTIVATE ~= (N + 625)/1.2 ns on HW vs the default 172-cycle init; DVE
# PSUM evacs measure ~(N + 380)/0.96).  With the default values the list
# scheduler believes the exp stream runs ~20% faster than it does, packs
# attention back-to-back, and defers projection matmuls past whole
# attention blocks — leaving the PE queue unable to fill real exp stalls.
hw_specs.TRN2Spec.ACCESS_CYCLES[
    (bass.MemorySpace.PSUM, mybir.EngineType.Activation)] = 625
hw_specs.TRN2Spec.ACCESS_CYCLES[
    (bass.MemorySpace.PSUM, mybir.EngineType.DVE)] = 380

B, S, D, H = 2, 2048, 1024, 16
DK = D // H                  # 64 head dim
NCORE = 8
GROUPS = NCORE // B          # 4 head-groups per batch
HPC = H // GROUPS            # 4 heads per core
FPC = HPC * DK               # 256 features per core
FT = FPC // 128              # 2 feature tiles (= head pairs) per core
DT = D // 128                # 8 d_in tiles
TT = S // 128                # 16 token tiles (k tiles)
QB = 512                     # query block (free-dim) size in attention
NQB = S // QB                # 4 query blocks = pipeline stages
BF = mybir.dt.bfloat16
F32 = mybir.dt.float32
BFNP = ml_dtypes.bfloat16

# module-level knobs for test.py
PROFILE = False
TRACE_CORES = None
LAST_RESULT = None

_program_cache: dict = {}


def _classify_mask(mask2d: np.ndarray):
    """Classify (S, S) keep-mask into per-(qblock, ktile) modes.

    Returns (plan, patterns): plan[qb] is a list of (kt, mask_id|None, c0, c1)
    for tiles that are at least partially kept, where c0 is the first
    q-column (within the block) with any kept key and [c0, c1) the columns
    needing the multiplicative mask; patterns[mid] is a [128, 2, c1-c0] bf16
    multiplicative mask (k on partitions, duplicated across the head axis).
    """
    keep = np.asarray(mask2d) != 0
    patterns = []
    pattern_ids = {}
    plan = []
    for qb in range(NQB):
        row = []
        for kt in range(TT):
            blk = keep[qb * QB:(qb + 1) * QB, kt * 128:(kt + 1) * 128].T
            if not blk.any():
                continue
            if blk.all():
                row.append((kt, None, 0, 0))
                continue
            anyk = blk.any(axis=0)
            allk = blk.all(axis=0)
            c0 = int(np.flatnonzero(anyk)[0])
            notall = np.flatnonzero(~allk)
            c1 = int(notall[-1]) + 1 if notall.size else c0
            pat = blk[:, c0:c1]
            key = pat.tobytes()
            mid = pattern_ids.get(key)
            if mid is None:
                mid = len(patterns)
                pattern_ids[key] = mid
                # duplicate across the 2-head axis of the pT tile
                patterns.append(np.repeat(pat[:, None, :], 2, axis=1)
                                .astype(BFNP))
            row.append((kt, mid, c0, c1))
        plan.append(row)
    return plan, patterns


def build_program(plan, npat, pw):
    nc = bacc.Bacc("TRN2", target_bir_lowering=False, debug=False,
                   num_devices=NCORE)
    qT = nc.dram_tensor("qT", (D, S), BF, kind="ExternalInput").ap()
    kT = nc.dram_tensor("kT", (D, S), BF, kind="ExternalInput").ap()
    vT = nc.dram_tensor("vT", (D, S), BF, kind="ExternalInput").ap()
    wqT = nc.dram_tensor("wqT", (D, FPC), BF, kind="ExternalInput").ap()
    wkT = nc.dram_tensor("wkT", (D, FPC), BF, kind="ExternalInput").ap()
    wvT = nc.dram_tensor("wvT", (D, FPC), BF, kind="ExternalInput").ap()
    woT = nc.dram_tensor("woT", (FPC, D), BF, kind="ExternalInput").ap()
    bqk = nc.dram_tensor("bqk", (2, FT, 128), F32, kind="ExternalInput").ap()
    masks = None
    if npat:
        masks = nc.dram_tensor("masks", (npat, 128, 2 * pw), BF,
                               kind="ExternalInput").ap()
    out = nc.dram_tensor("out", (S, D), BF, kind="ExternalOutput").ap()

    with tile.TileContext(nc) as tc, ExitStack() as ctx:
        singles = ctx.enter_context(tc.tile_pool(name="singles", bufs=1))
        xpool = ctx.enter_context(tc.tile_pool(name="xpool", bufs=2))
        ppool = ctx.enter_context(tc.tile_pool(name="ppool", bufs=4))
        npool = ctx.enter_context(tc.tile_pool(name="npool", bufs=2))
        upool = ctx.enter_context(tc.tile_pool(name="upool", bufs=2))
        opool = ctx.enter_context(tc.tile_pool(name="opool", bufs=2))
        psS = ctx.enter_context(tc.tile_pool(name="psS", bufs=2, space="PSUM"))
        psPV = ctx.enter_context(tc.tile_pool(name="psPV", bufs=1, space="PSUM"))

        # ---- SBUF residents ----
        wq_sb = singles.tile([128, DT, FPC], BF)
        wk_sb = singles.tile([128, DT, FPC], BF)
        wv_sb = singles.tile([128, DT, FPC], BF)
        wo_sb = singles.tile([128, FT, D], BF)
        bias_sb = singles.tile([128, 2, FT], F32)
        mask_sb = None
        if npat:
            mask_sb = singles.tile([128, npat, 2 * pw], BF, name="mask_sb")
        q_sb = singles.tile([128, FT, S], BF)
        k_sb = singles.tile([128, FT, S], BF)
        attn_sb = singles.tile([128, FT, S], BF)
        v_sb = singles.tile([128, TT, HPC, DK + 1], BF)

        # ---- input DMA issue order (sync queue, FIFO = priority): the
        # first compute (K0/Q0 projection, then the exp stream) needs
        # wk+xk0+wq+xq0, so those go first ----
        xk_c = [xpool.tile([128, DT, QB], BF, name=f"xk{c}", tag="xk")
                for c in range(NQB)]
        xq_c = [xpool.tile([128, DT, QB], BF, name=f"xq{c}", tag="xq")
                for c in range(NQB)]
        xv_c = [xpool.tile([128, DT, QB], BF, name=f"xv{c}", tag="xv")
                for c in range(NQB)]

        def chunk_dma(x_sb, x_dram, c):
            nc.sync.dma_start(
                x_sb,
                x_dram[:, c * QB:(c + 1) * QB]
                .rearrange("(t p) f -> p t f", p=128))

        # issue order on the sync ring = first-need order: K0/Q0 projection
        # inputs first (they gate the exp stream), then V0, then the rest
        nc.sync.dma_start(bias_sb, bqk.rearrange("a b p -> p a b"))
        nc.sync.dma_start(wk_sb, wkT.rearrange("(t p) f -> p t f", p=128))
        chunk_dma(xk_c[0], kT, 0)
        nc.sync.dma_start(wq_sb, wqT.rearrange("(t p) f -> p t f", p=128))
        chunk_dma(xq_c[0], qT, 0)
        nc.sync.dma_start(wv_sb, wvT.rearrange("(t p) f -> p t f", p=128))
        chunk_dma(xv_c[0], vT, 0)
        if npat:
            nc.sync.dma_start(mask_sb, masks.rearrange("m p f -> p m f"))
        nc.sync.dma_start(wo_sb, woT.rearrange("(t p) f -> p t f", p=128))
        for c in range(1, NQB):
            chunk_dma(xk_c[c], kT, c)
            chunk_dma(xq_c[c], qT, c)
            chunk_dma(xv_c[c], vT, c)

        # trailing ones column of V: PV's last output partition (64 — still
        # 32-aligned for VectorE access) accumulates the softmax denominator
        for tt in range(TT):
            nc.vector.memset(v_sb[:, tt, :, DK:DK + 1], 1.0)

        # PE warm-up: ~6us of dummy matmuls (one accumulation group, so only
        # one PSUM ring slot is held) while the first inputs stream in, so the
        # HAM clock gate is at 8/8 when the real work starts (and doesn't
        # re-throttle before the first projection's inputs land).
        warm_sb = singles.tile([128, QB], BF)
        nc.vector.memset(warm_sb, 0.0)
        ps = psS.tile([128, QB], F32, tag="acc", name="warm_ps")
        for r in range(14):
            nc.tensor.matmul(ps, lhsT=warm_sb[:, 0:128], rhs=warm_sb,
                             start=(r == 0), stop=(r == 13))

        inv_sqrt_dk = float(1.0 / np.sqrt(DK))

        def o_proj(qb):
            # partial O-projection for query block qb (deferred one stage so
            # the PE never waits on qb's normalize chain)
            ob = opool.tile([128, QB // 128, D], BF, name="ob")
            for qt in range(QB // 128):
                for nch in range(D // QB):
                    ps = psS.tile([128, QB], F32, tag="acc", name="ps")
                    for hd in range(FT):
                        nc.tensor.matmul(
                            ps,
                            lhsT=attn_sb[:, hd,
                                         qb * QB + qt * 128:
                                         qb * QB + (qt + 1) * 128],
                            rhs=wo_sb[:, hd, nch * QB:(nch + 1) * QB],
                            start=(hd == 0), stop=(hd == FT - 1))
                    nc.vector.tensor_copy(ob[:, qt, nch * QB:(nch + 1) * QB],
                                          ps)
                if qt % 2 == 1:
                    # write out each half as soon as its evacs land
                    nc.sync.dma_start(
                        out[qb * QB + (qt - 1) * 128:
                            qb * QB + (qt + 1) * 128, :]
                        .rearrange("(t p) f -> p t f", p=128),
                        ob[:, qt - 1:qt + 1, :])

        def kq_proj(c, ft):
            # K^T / Q^T projection for chunk c, one feature tile
            # ([feat, tok] layout; the evac adds the bias from PSUM)
            for bi, x_sb, w_sb, y_sb in ((1, xk_c[c], wk_sb, k_sb),
                                         (0, xq_c[c], wq_sb, q_sb)):
                ps = psS.tile([128, QB], F32, tag="acc", name="ps")
                for dt in range(DT):
                    nc.tensor.matmul(
                        ps,
                        lhsT=w_sb[:, dt, ft * 128:(ft + 1) * 128],
                        rhs=x_sb[:, dt, :],
                        start=(dt == 0), stop=(dt == DT - 1))
                nc.vector.tensor_scalar_add(
                    y_sb[:, ft, c * QB:(c + 1) * QB], ps,
                    bias_sb[:, bi, ft:ft + 1])

        def v_proj(c):
            # V projection for chunk c ([tok, feat] layout)
            for tt in range(4 * c, 4 * c + 4):
                ps = psS.tile([128, FPC], F32, tag="acc", name="ps")
                for dt in range(DT):
                    nc.tensor.matmul(
                        ps,
                        lhsT=xv_c[c][:, dt, (tt - 4 * c) * 128:
                                     (tt - 4 * c + 1) * 128],
                        rhs=wv_sb[:, dt, :],
                        start=(dt == 0), stop=(dt == DT - 1))
                nc.vector.tensor_copy(
                    v_sb[:, tt, :, 0:DK],
                    ps.rearrange("p (h d) -> p h d", h=HPC))

        def attention(qb, ft):
            # causal attention for query block qb, head pair ft
            kts = plan[qb]
            hA, hB = 2 * ft, 2 * ft + 1
            pv = psPV.tile([DK + 1, 2, QB], F32, name="pv")
            n = len(kts)
            pts = []
            for i, (kt, mid, c0, c1) in enumerate(kts):
                # QK^T for both heads: row groups 0-1 / 2-3, concurrent
                s = psS.tile([128, 2, QB], F32, tag="qk", name="s")
                for j, pr in ((0, 0), (1, 64)):
                    nc.tensor.matmul(
                        s[:, j, c0:],
                        lhsT=k_sb[pr:pr + DK, ft,
                                  kt * 128:(kt + 1) * 128],
                        rhs=q_sb[pr:pr + DK, ft,
                                 qb * QB + c0:(qb + 1) * QB],
                        start=True, stop=True)
                pT = ppool.tile([128, 2, QB], BF, tag="pt", name="pT")
                nc.scalar.activation(pT[:, :, c0:], s[:, :, c0:],
                                     mybir.ActivationFunctionType.Exp,
                                     scale=inv_sqrt_dk)
                if mid is not None and c1 > c0:
                    assert mask_sb is not None
                    w = c1 - c0
                    nc.vector.tensor_mul(
                        pT[:, :, c0:c1], pT[:, :, c0:c1],
                        mask_sb[:, mid, 0:2 * w].rearrange(
                            "p (j w) -> p j w", j=2))
                pts.append((kt, c0, pT))
                # software pipeline: PV lags one k-tile behind QK/exp
                if i > 0:
                    pkt, pc0, ppT = pts[i - 1]
                    for j, h in ((0, hA), (1, hB)):
                        nc.tensor.matmul(
                            pv[:, j, pc0:], lhsT=v_sb[:, pkt, h, :],
                            rhs=ppT[:, j, pc0:],
                            start=(i == 1), stop=False)
            pkt, pc0, ppT = pts[n - 1]
            for j, h in ((0, hA), (1, hB)):
                nc.tensor.matmul(
                    pv[:, j, pc0:], lhsT=v_sb[:, pkt, h, :],
                    rhs=ppT[:, j, pc0:],
                    start=(n == 1), stop=True)

            # normalize per column half (shorter PSUM hold, and the next
            # consumer can start on the first half earlier):
            # attn^T[d, q] = attnU^T[d, q] / denom[q], denom = pv partition 0
            HB2 = QB // 2
            for lo in (0, HB2):
                # ScalarE moves the denominator to partition 0 (the only
                # engine that can shift partitions; custom DVE ops require
                # base partition 0), then DVE reciprocal + GpSimd broadcast
                den = npool.tile([1, 2, HB2], F32, tag="den", name="den")
                nc.scalar.copy(den, pv[DK:DK + 1, :, lo:lo + HB2])
                rec = npool.tile([1, 2, HB2], F32, tag="rec", name="rec")
                nc.vector.reciprocal_approx_fast(rec, den)
                bc = npool.tile([DK, 2, HB2], F32, tag="bc", name="bc")
                nc.gpsimd.partition_broadcast(bc, rec)
                qcols = slice(qb * QB + lo, qb * QB + lo + HB2)
                nc.vector.tensor_mul(attn_sb[0:DK, ft, qcols],
                                     pv[0:DK, 0, lo:lo + HB2],
                                     bc[:, 0, :])
                nc.vector.tensor_mul(attn_sb[64:64 + DK, ft, qcols],
                                     pv[0:DK, 1, lo:lo + HB2],
                                     bc[:, 1, :])

        # Stage pipeline.  The attention stream (QK -> exp -> PV) is
        # high-priority: the scheduler fills its pipeline stalls with
        # projection / O-projection matmuls instead of stalling the PE
        # queue.  Attention blocks run biggest-block-first at the end —
        # (0, ft=1), which needs only chunk 0, runs LAST so qb3's PV /
        # normalize / O-projection hide under its exp stream and the
        # kernel tail is the smallest block's.
        kq_proj(0, 0)
        v_proj(0)
        with tc.high_priority():
            attention(0, 0)
        kq_proj(0, 1)
        for c in range(1, NQB):
            kq_proj(c, 0)
            v_proj(c)
            with tc.high_priority():
                attention(c, 0)
            if c >= 2:
                o_proj(c - 1)
            kq_proj(c, 1)
            with tc.high_priority():
                attention(c, 1)
        with tc.high_priority():
            attention(0, 1)
        o_proj(NQB - 1)
        o_proj(0)

    nc.compile()
    return nc


def _get_program(mask2d: np.ndarray):
    key = hashlib.sha1(np.ascontiguousarray(mask2d).tobytes()).hexdigest()
    hit = _program_cache.get(key)
    if hit is not None:
        return hit
    plan, patterns = _classify_mask(mask2d)
    pw = max((p.shape[2] for p in patterns), default=0)
    nc = build_program(plan, len(patterns), pw)
    if patterns:
        pat = np.zeros((len(patterns), 128, 2 * pw), BFNP)
        for i, p in enumerate(patterns):
            w = p.shape[2]
            pat[i, :, :2 * w] = p.reshape(128, 2 * w)
    else:
        pat = None
    _program_cache[key] = (nc, pat)
    return nc, pat


def kernel(**inputs) -> np.ndarray:
    global LAST_RESULT
    query = np.asarray(inputs["query"], np.float32)
    key = np.asarray(inputs["key"], np.float32)
    value = np.asarray(inputs["value"], np.float32)
    mask = np.asarray(inputs["mask"])
    Wq = np.asarray(inputs["Wq"], np.float32)
    bq = np.asarray(inputs["bq"], np.float32)
    Wk = np.asarray(inputs["Wk"], np.float32)
    bk = np.asarray(inputs["bk"], np.float32)
    Wv = np.asarray(inputs["Wv"], np.float32)
    bv = np.asarray(inputs["bv"], np.float32)
    Wo = np.asarray(inputs["Wo"], np.float32)
    bo = np.asarray(inputs["bo"], np.float32)

    nc, pat = _get_program(mask.reshape(S, S))

    WqT, WkT, WvT, WoT = Wq.T, Wk.T, Wv.T, Wo.T
    xT = {
        t: [np.ascontiguousarray(x[b].T).astype(BFNP) for b in range(B)]
        for t, x in (("qT", query), ("kT", key), ("vT", value))
    }
    in_maps = []
    for c in range(NCORE):
        b, g = divmod(c, GROUPS)
        f0 = g * FPC
        m = {
            "qT": xT["qT"][b],
            "kT": xT["kT"][b],
            "vT": xT["vT"][b],
            "wqT": np.ascontiguousarray(WqT[:, f0:f0 + FPC]).astype(BFNP),
            "wkT": np.ascontiguousarray(WkT[:, f0:f0 + FPC]).astype(BFNP),
            "wvT": np.ascontiguousarray(WvT[:, f0:f0 + FPC]).astype(BFNP),
            "woT": np.ascontiguousarray(WoT[f0:f0 + FPC, :]).astype(BFNP),
            "bqk": np.stack([bq[f0:f0 + FPC].reshape(FT, 128),
                             bk[f0:f0 + FPC].reshape(FT, 128)]).astype(np.float32),
        }
        if pat is not None:
            m["masks"] = pat
        in_maps.append(m)

    res = run_bass_kernel_spmd(
        nc, in_maps, core_ids=list(range(NCORE)),
        trace=PROFILE,
        trace_cores=(TRACE_CORES if TRACE_CORES is not None
                     else (list(range(NCORE)) if PROFILE else None)),
    )
    LAST_RESULT = res

    host_bias = bo + bv @ WoT  # (D,) folded V/O biases, added once per batch
    out = np.empty((B, S, D), np.float32)
    for b in range(B):
        acc = res.results[b * GROUPS]["out"].astype(np.float32)
        for g in range(1, GROUPS):
            acc = acc + res.results[b * GROUPS + g]["out"].astype(np.float32)
        out[b] = acc + host_bias
    return out


# revision 27
# speedup vs baseline: 1.2360x; 1.2360x over previous
"""Multi-head attention (B=2, S=2048, D=1024, H=16) on 8 Trainium2 NeuronCores.

Sharding: core c -> (batch b = c//4, head-group g = c%4).  Each core computes
Q/K/V projections for its 4 heads (256 features), causal attention for those
heads over the full sequence, and a partial O-projection (its 256 attn
features x full Wo.T slice).  The host sums the 4 bf16 partial outputs per
batch and folds in the biases that commute with the reduction (bo, bv @ Wo.T).

Pipeline (per core): the sequence is processed in 4 chunks of 512 tokens.
Stage c projects K/Q/V for chunk c, then runs causal attention for query
block c (which only needs K/V chunks 0..c), then the partial O-projection
for that block.  This starts the ScalarE exp stream (the serial bottleneck)
~45us earlier than projecting everything upfront.

Engine assignment:
  TensorE   projections + QK^T + PV + O-proj.  QK^T contracts over dk=64, so
            the two heads of a feature tile (partitions 0-63 / 64-127) issue
            back-to-back and run concurrently in different PE row groups
            (tile_position auto-derived from base partitions).
  ScalarE   only exp (fused 1/sqrt(dk) scale), one ACTIVATE per k-tile
            covering both heads of the pair via a [128, 2, QB] PSUM tile.
  VectorE   PSUM evacuations (+ K/Q bias add), mask multiplies, reciprocal,
            normalize multiplies.
  GpSimd    partition-broadcast of 1/denom; output DMA (SWDGE) so the Sync
            queue stays dedicated to input streaming.
  V layout  [tok, feat+1] with a ones column: PV also accumulates the
            softmax denominator (scores are O(5): no max subtraction needed,
            f32 exp cannot overflow).
"""

import hashlib
from contextlib import ExitStack

import ml_dtypes
import numpy as np

import concourse.bass as bass
import concourse.tile as tile
from concourse import bacc, hw_specs, mybir
from concourse.bass_utils import run_bass_kernel_spmd

# Calibrate the Tile scheduler's cost model to measured slice durations
# (ACTIVATE ~= (N + 625)/1.2 ns on HW vs the default 172-cycle init; DVE
# PSUM evacs measure ~(N + 380)/0.96).  With the default values the list
# scheduler believes the exp stream runs ~20% faster than it does, packs
# attention back-to-back, and defers projection matmuls past whole
# attention blocks — leaving the PE queue unable to fill real exp stalls.
hw_specs.TRN2Spec.ACCESS_CYCLES[
    (bass.MemorySpace.PSUM, mybir.EngineType.Activation)] = 625
hw_specs.TRN2Spec.ACCESS_CYCLES[
    (bass.MemorySpace.PSUM, mybir.EngineType.DVE)] = 380

B, S, D, H = 2, 2048, 1024, 16
DK = D // H                  # 64 head dim
NCORE = 8
GROUPS = NCORE // B          # 4 head-groups per batch
HPC = H // GROUPS            # 4 heads per core
FPC = HPC * DK               # 256 features per core
FT = FPC // 128              # 2 feature tiles (= head pairs) per core
DT = D // 128                # 8 d_in tiles
TT = S // 128                # 16 token tiles (k tiles)
QB = 512                     # query block (free-dim) size in attention
NQB = S // QB                # 4 query blocks = pipeline stages
BF = mybir.dt.bfloat16
F32 = mybir.dt.float32
BFNP = ml_dtypes.bfloat16

# module-level knobs for test.py
PROFILE = False
TRACE_CORES = None
LAST_RESULT = None

_program_cache: dict = {}


def _classify_mask(mask2d: np.ndarray):
    """Classify (S, S) keep-mask into per-(qblock, ktile) modes.

    Returns (plan, patterns): plan[qb] is a list of (kt, mask_id|None, c0, c1)
    for tiles that are at least partially kept, where c0 is the first
    q-column (within the block) with any kept key and [c0, c1) the columns
    needing the multiplicative mask; patterns[mid] is a [128, 2, c1-c0] bf16
    multiplicative mask (k on partitions, duplicated across the head axis).
    """
    keep = np.asarray(mask2d) != 0
    patterns = []
    pattern_ids = {}
    plan = []
    for qb in range(NQB):
        row = []
        for kt in range(TT):
            blk = keep[qb * QB:(qb + 1) * QB, kt * 128:(kt + 1) * 128].T
            if not blk.any():
                continue
            if blk.all():
                row.append((kt, None, 0, 0))
                continue
            anyk = blk.any(axis=0)
            allk = blk.all(axis=0)
            c0 = int(np.flatnonzero(anyk)[0])
            notall = np.flatnonzero(~allk)
            c1 = int(notall[-1]) + 1 if notall.size else c0
            pat = blk[:, c0:c1]
            key = pat.tobytes()
            mid = pattern_ids.get(key)
            if mid is None:
                mid = len(patterns)
                pattern_ids[key] = mid
                # duplicate across the 2-head axis of the pT tile
                patterns.append(np.repeat(pat[:, None, :], 2, axis=1)
                                .astype(BFNP))
            row.append((kt, mid, c0, c1))
        plan.append(row)
    return plan, patterns


def build_program(plan, npat, pw):
    nc = bacc.Bacc("TRN2", target_bir_lowering=False, debug=False,
                   num_devices=NCORE)
    qT = nc.dram_tensor("qT", (D, S), BF, kind="ExternalInput").ap()
    kT = nc.dram_tensor("kT", (D, S), BF, kind="ExternalInput").ap()
    vT = nc.dram_tensor("vT", (D, S), BF, kind="ExternalInput").ap()
    wqT = nc.dram_tensor("wqT", (D, FPC), BF, kind="ExternalInput").ap()
    wkT = nc.dram_tensor("wkT", (D, FPC), BF, kind="ExternalInput").ap()
    wvT = nc.dram_tensor("wvT", (D, FPC), BF, kind="ExternalInput").ap()
    woT = nc.dram_tensor("woT", (FPC, D), BF, kind="ExternalInput").ap()
    bqk = nc.dram_tensor("bqk", (2, FT, 128), F32, kind="ExternalInput").ap()
    masks = None
    if npat:
        masks = nc.dram_tensor("masks", (npat, 128, 2 * pw), BF,
                               kind="ExternalInput").ap()
    out = nc.dram_tensor("out", (S, D), BF, kind="ExternalOutput").ap()

    with tile.TileContext(nc) as tc, ExitStack() as ctx:
        singles = ctx.enter_context(tc.tile_pool(name="singles", bufs=1))
        xpool = ctx.enter_context(tc.tile_pool(name="xpool", bufs=2))
        ppool = ctx.enter_context(tc.tile_pool(name="ppool", bufs=4))
        npool = ctx.enter_context(tc.tile_pool(name="npool", bufs=2))
        upool = ctx.enter_context(tc.tile_pool(name="upool", bufs=2))
        opool = ctx.enter_context(tc.tile_pool(name="opool", bufs=2))
        psS = ctx.enter_context(tc.tile_pool(name="psS", bufs=2, space="PSUM"))
        psPV = ctx.enter_context(tc.tile_pool(name="psPV", bufs=1, space="PSUM"))

        # ---- SBUF residents ----
        wq_sb = singles.tile([128, DT, FPC], BF)
        wk_sb = singles.tile([128, DT, FPC], BF)
        wv_sb = singles.tile([128, DT, FPC], BF)
        wo_sb = singles.tile([128, FT, D], BF)
        bias_sb = singles.tile([128, 2, FT], F32)
        mask_sb = None
        if npat:
            mask_sb = singles.tile([128, npat, 2 * pw], BF, name="mask_sb")
        q_sb = singles.tile([128, FT, S], BF)
        k_sb = singles.tile([128, FT, S], BF)
        attn_sb = singles.tile([128, FT, S], BF)
        v_sb = singles.tile([128, TT, HPC, DK + 1], BF)

        # ---- input DMA issue order (sync queue, FIFO = priority): the
        # first compute (K0/Q0 projection, then the exp stream) needs
        # wk+xk0+wq+xq0, so those go first ----
        xk_c = [xpool.tile([128, DT, QB], BF, name=f"xk{c}", tag="xk")
                for c in range(NQB)]
        xq_c = [xpool.tile([128, DT, QB], BF, name=f"xq{c}", tag="xq")
                for c in range(NQB)]
        xv_c = [xpool.tile([128, DT, QB], BF, name=f"xv{c}", tag="xv")
                for c in range(NQB)]

        def chunk_dma(x_sb, x_dram, c):
            nc.sync.dma_start(
                x_sb,
                x_dram[:, c * QB:(c + 1) * QB]
                .rearrange("(t p) f -> p t f", p=128))

        # issue order on the sync ring = first-need order: K0/Q0 projection
        # inputs first (they gate the exp stream), then V0, then the rest
        nc.sync.dma_start(bias_sb, bqk.rearrange("a b p -> p a b"))
        nc.sync.dma_start(wk_sb, wkT.rearrange("(t p) f -> p t f", p=128))
        chunk_dma(xk_c[0], kT, 0)
        nc.sync.dma_start(wq_sb, wqT.rearrange("(t p) f -> p t f", p=128))
        chunk_dma(xq_c[0], qT, 0)
        nc.sync.dma_start(wv_sb, wvT.rearrange("(t p) f -> p t f", p=128))
        chunk_dma(xv_c[0], vT, 0)
        if npat:
            nc.sync.dma_start(mask_sb, masks.rearrange("m p f -> p m f"))
        nc.sync.dma_start(wo_sb, woT.rearrange("(t p) f -> p t f", p=128))
        for c in range(1, NQB):
            chunk_dma(xk_c[c], kT, c)
            chunk_dma(xq_c[c], qT, c)
            chunk_dma(xv_c[c], vT, c)

        # trailing ones column of V: PV's last output partition (64 — still
        # 32-aligned for VectorE access) accumulates the softmax denominator
        for tt in range(TT):
            nc.vector.memset(v_sb[:, tt, :, DK:DK + 1], 1.0)

        # PE warm-up: ~6us of dummy matmuls (one accumulation group, so only
        # one PSUM ring slot is held) while the first inputs stream in, so the
        # HAM clock gate is at 8/8 when the real work starts (and doesn't
        # re-throttle before the first projection's inputs land).
        warm_sb = singles.tile([128, QB], BF)
        nc.vector.memset(warm_sb, 0.0)
        ps = psS.tile([128, QB], F32, tag="acc", name="warm_ps")
        for r in range(14):
            nc.tensor.matmul(ps, lhsT=warm_sb[:, 0:128], rhs=warm_sb,
                             start=(r == 0), stop=(r == 13))

        inv_sqrt_dk = float(1.0 / np.sqrt(DK))

        def o_proj(qb):
            # partial O-projection for query block qb (deferred one stage so
            # the PE never waits on qb's normalize chain)
            ob = opool.tile([128, QB // 128, D], BF, name="ob")
            for qt in range(QB // 128):
                for nch in range(D // QB):
                    ps = psS.tile([128, QB], F32, tag="acc", name="ps")
                    for hd in range(FT):
                        nc.tensor.matmul(
                            ps,
                            lhsT=attn_sb[:, hd,
                                         qb * QB + qt * 128:
                                         qb * QB + (qt + 1) * 128],
                            rhs=wo_sb[:, hd, nch * QB:(nch + 1) * QB],
                            start=(hd == 0), stop=(hd == FT - 1))
                    nc.vector.tensor_copy(ob[:, qt, nch * QB:(nch + 1) * QB],
                                          ps)
                if qt % 2 == 1:
                    # write out each half as soon as its evacs land
                    nc.sync.dma_start(
                        out[qb * QB + (qt - 1) * 128:
                            qb * QB + (qt + 1) * 128, :]
                        .rearrange("(t p) f -> p t f", p=128),
                        ob[:, qt - 1:qt + 1, :])

        def kq_proj(c, ft):
            # K^T / Q^T projection for chunk c, one feature tile
            # ([feat, tok] layout; the evac adds the bias from PSUM)
            for bi, x_sb, w_sb, y_sb in ((1, xk_c[c], wk_sb, k_sb),
                                         (0, xq_c[c], wq_sb, q_sb)):
                ps = psS.tile([128, QB], F32, tag="acc", name="ps")
                for dt in range(DT):
                    nc.tensor.matmul(
                        ps,
                        lhsT=w_sb[:, dt, ft * 128:(ft + 1) * 128],
                        rhs=x_sb[:, dt, :],
                        start=(dt == 0), stop=(dt == DT - 1))
                nc.vector.tensor_scalar_add(
                    y_sb[:, ft, c * QB:(c + 1) * QB], ps,
                    bias_sb[:, bi, ft:ft + 1])

        def v_proj(c):
            # V projection for chunk c ([tok, feat] layout)
            for tt in range(4 * c, 4 * c + 4):
                ps = psS.tile([128, FPC], F32, tag="acc", name="ps")
                for dt in range(DT):
                    nc.tensor.matmul(
                        ps,
                        lhsT=xv_c[c][:, dt, (tt - 4 * c) * 128:
                                     (tt - 4 * c + 1) * 128],
                        rhs=wv_sb[:, dt, :],
                        start=(dt == 0), stop=(dt == DT - 1))
                nc.vector.tensor_copy(
                    v_sb[:, tt, :, 0:DK],
                    ps.rearrange("p (h d) -> p h d", h=HPC))

        def attention(qb, ft):
            # causal attention for query block qb, head pair ft
            kts = plan[qb]
            hA, hB = 2 * ft, 2 * ft + 1
            pv = psPV.tile([DK + 1, 2, QB], F32, name="pv")
            n = len(kts)
            pts = []
            for i, (kt, mid, c0, c1) in enumerate(kts):
                # QK^T for both heads: row groups 0-1 / 2-3, concurrent
                s = psS.tile([128, 2, QB], F32, tag="qk", name="s")
                for j, pr in ((0, 0), (1, 64)):
                    nc.tensor.matmul(
                        s[:, j, c0:],
                        lhsT=k_sb[pr:pr + DK, ft,
                                  kt * 128:(kt + 1) * 128],
                        rhs=q_sb[pr:pr + DK, ft,
                                 qb * QB + c0:(qb + 1) * QB],
                        start=True, stop=True)
                pT = ppool.tile([128, 2, QB], BF, tag="pt", name="pT")
                nc.scalar.activation(pT[:, :, c0:], s[:, :, c0:],
                                     mybir.ActivationFunctionType.Exp,
                                     scale=inv_sqrt_dk)
                if mid is not None and c1 > c0:
                    assert mask_sb is not None
                    w = c1 - c0
                    nc.vector.tensor_mul(
                        pT[:, :, c0:c1], pT[:, :, c0:c1],
                        mask_sb[:, mid, 0:2 * w].rearrange(
                            "p (j w) -> p j w", j=2))
                pts.append((kt, c0, pT))
                # software pipeline: PV lags one k-tile behind QK/exp
                if i > 0:
                    pkt, pc0, ppT = pts[i - 1]
                    for j, h in ((0, hA), (1, hB)):
                        nc.tensor.matmul(
                            pv[:, j, pc0:], lhsT=v_sb[:, pkt, h, :],
                            rhs=ppT[:, j, pc0:],
                            start=(i == 1), stop=False)
            pkt, pc0, ppT = pts[n - 1]
            for j, h in ((0, hA), (1, hB)):
                nc.tensor.matmul(
                    pv[:, j, pc0:], lhsT=v_sb[:, pkt, h, :],
                    rhs=ppT[:, j, pc0:],
                    start=(n == 1), stop=True)

            # normalize per column half (shorter PSUM hold, and the next
            # consumer can start on the first half earlier):
            # attn^T[d, q] = attnU^T[d, q] / denom[q], denom = pv partition 0
            HB2 = QB // 2
            for lo in (0, HB2):
                # ScalarE moves the denominator to partition 0 (the only
                # engine that can shift partitions; custom DVE ops require
                # base partition 0), then DVE reciprocal + GpSimd broadcast
                den = npool.tile([1, 2, HB2], F32, tag="den", name="den")
                nc.scalar.copy(den, pv[DK:DK + 1, :, lo:lo + HB2])
                u = upool.tile([DK, 2, HB2], BF, tag="u", name="u")
                nc.vector.tensor_copy(u, pv[0:DK, :, lo:lo + HB2])
                rec = npool.tile([1, 2, HB2], F32, tag="rec", name="rec")
                nc.vector.reciprocal_approx_fast(rec, den)
                bc = npool.tile([DK, 2, HB2], F32, tag="bc", name="bc")
                nc.gpsimd.partition_broadcast(bc, rec)
                qcols = slice(qb * QB + lo, qb * QB + lo + HB2)
                nc.vector.tensor_mul(attn_sb[0:DK, ft, qcols],
                                     u[:, 0, :], bc[:, 0, :])
                nc.vector.tensor_mul(attn_sb[64:64 + DK, ft, qcols],
                                     u[:, 1, :], bc[:, 1, :])

        # Stage pipeline.  The attention stream (QK -> exp -> PV) is
        # high-priority: the scheduler fills its pipeline stalls with
        # projection / O-projection matmuls instead of stalling the PE
        # queue.  Attention blocks run biggest-block-first at the end —
        # (0, ft=1), which needs only chunk 0, runs LAST so qb3's PV /
        # normalize / O-projection hide under its exp stream and the
        # kernel tail is the smallest block's.
        kq_proj(0, 0)
        v_proj(0)
        with tc.high_priority():
            attention(0, 0)
        kq_proj(0, 1)
        for c in range(1, NQB):
            kq_proj(c, 0)
            v_proj(c)
            with tc.high_priority():
                attention(c, 0)
            if c >= 2:
                o_proj(c - 1)
            kq_proj(c, 1)
            with tc.high_priority():
                attention(c, 1)
        with tc.high_priority():
            attention(0, 1)
        o_proj(NQB - 1)
        o_proj(0)

    nc.compile()
    return nc


def _get_program(mask2d: np.ndarray):
    key = hashlib.sha1(np.ascontiguousarray(mask2d).tobytes()).hexdigest()
    hit = _program_cache.get(key)
    if hit is not None:
        return hit
    plan, patterns = _classify_mask(mask2d)
    pw = max((p.shape[2] for p in patterns), default=0)
    nc = build_program(plan, len(patterns), pw)
    if patterns:
        pat = np.zeros((len(patterns), 128, 2 * pw), BFNP)
        for i, p in enumerate(patterns):
            w = p.shape[2]
            pat[i, :, :2 * w] = p.reshape(128, 2 * w)
    else:
        pat = None
    _program_cache[key] = (nc, pat)
    return nc, pat


def kernel(**inputs) -> np.ndarray:
    global LAST_RESULT
    query = np.asarray(inputs["query"], np.float32)
    key = np.asarray(inputs["key"], np.float32)
    value = np.asarray(inputs["value"], np.float32)
    mask = np.asarray(inputs["mask"])
    Wq = np.asarray(inputs["Wq"], np.float32)
    bq = np.asarray(inputs["bq"], np.float32)
    Wk = np.asarray(inputs["Wk"], np.float32)
    bk = np.asarray(inputs["bk"], np.float32)
    Wv = np.asarray(inputs["Wv"], np.float32)
    bv = np.asarray(inputs["bv"], np.float32)
    Wo = np.asarray(inputs["Wo"], np.float32)
    bo = np.asarray(inputs["bo"], np.float32)

    nc, pat = _get_program(mask.reshape(S, S))

    WqT, WkT, WvT, WoT = Wq.T, Wk.T, Wv.T, Wo.T
    xT = {
        t: [np.ascontiguousarray(x[b].T).astype(BFNP) for b in range(B)]
        for t, x in (("qT", query), ("kT", key), ("vT", value))
    }
    in_maps = []
    for c in range(NCORE):
        b, g = divmod(c, GROUPS)
        f0 = g * FPC
        m = {
            "qT": xT["qT"][b],
            "kT": xT["kT"][b],
            "vT": xT["vT"][b],
            "wqT": np.ascontiguousarray(WqT[:, f0:f0 + FPC]).astype(BFNP),
            "wkT": np.ascontiguousarray(WkT[:, f0:f0 + FPC]).astype(BFNP),
            "wvT": np.ascontiguousarray(WvT[:, f0:f0 + FPC]).astype(BFNP),
            "woT": np.ascontiguousarray(WoT[f0:f0 + FPC, :]).astype(BFNP),
            "bqk": np.stack([bq[f0:f0 + FPC].reshape(FT, 128),
                             bk[f0:f0 + FPC].reshape(FT, 128)]).astype(np.float32),
        }
        if pat is not None:
            m["masks"] = pat
        in_maps.append(m)

    res = run_bass_kernel_spmd(
        nc, in_maps, core_ids=list(range(NCORE)),
        trace=PROFILE,
        trace_cores=(TRACE_CORES if TRACE_CORES is not None
                     else (list(range(NCORE)) if PROFILE else None)),
    )
    LAST_RESULT = res

    host_bias = bo + bv @ WoT  # (D,) folded V/O biases, added once per batch
    out = np.empty((B, S, D), np.float32)
    for b in range(B):
        acc = res.results[b * GROUPS]["out"].astype(np.float32)
        for g in range(1, GROUPS):
            acc = acc + res.results[b * GROUPS + g]["out"].astype(np.float32)
        out[b] = acc + host_bias
    return out


# revision 28
# speedup vs baseline: 1.2436x; 1.0061x over previous
"""Multi-head attention (B=2, S=2048, D=1024, H=16) on 8 Trainium2 NeuronCores.

Sharding: core c -> (batch b = c//4, head-group g = c%4).  Each core computes
Q/K/V projections for its 4 heads (256 features), causal attention for those
heads over the full sequence, and a partial O-projection (its 256 attn
features x full Wo.T slice).  The host sums the 4 bf16 partial outputs per
batch and folds in the biases that commute with the reduction (bo, bv @ Wo.T).

Pipeline (per core): the sequence is processed in 4 chunks of 512 tokens.
Stage c projects K/Q/V for chunk c, then runs causal attention for query
block c (which only needs K/V chunks 0..c), then the partial O-projection
for that block.  This starts the ScalarE exp stream (the serial bottleneck)
~45us earlier than projecting everything upfront.

Engine assignment:
  TensorE   projections + QK^T + PV + O-proj.  QK^T contracts over dk=64, so
            the two heads of a feature tile (partitions 0-63 / 64-127) issue
            back-to-back and run concurrently in different PE row groups
            (tile_position auto-derived from base partitions).
  ScalarE   only exp (fused 1/sqrt(dk) scale), one ACTIVATE per k-tile
            covering both heads of the pair via a [128, 2, QB] PSUM tile.
  VectorE   PSUM evacuations (+ K/Q bias add), mask multiplies, reciprocal,
            normalize multiplies.
  GpSimd    partition-broadcast of 1/denom; output DMA (SWDGE) so the Sync
            queue stays dedicated to input streaming.
  V layout  [tok, feat+1] with a ones column: PV also accumulates the
            softmax denominator (scores are O(5): no max subtraction needed,
            f32 exp cannot overflow).
"""

import hashlib
from contextlib import ExitStack

import ml_dtypes
import numpy as np

import concourse.bass as bass
import concourse.tile as tile
from concourse import bacc, hw_specs, mybir
from concourse.bass_utils import run_bass_kernel_spmd

# Calibrate the Tile scheduler's cost model to measured slice durations
# (ACTIVATE ~= (N + 625)/1.2 ns on HW vs the default 172-cycle init; DVE
# PSUM evacs measure ~(N + 380)/0.96).  With the default values the list
# scheduler believes the exp stream runs ~20% faster than it does, packs
# attention back-to-back, and defers projection matmuls past whole
# attention blocks — leaving the PE queue unable to fill real exp stalls.
hw_specs.TRN2Spec.ACCESS_CYCLES[
    (bass.MemorySpace.PSUM, mybir.EngineType.Activation)] = 625
hw_specs.TRN2Spec.ACCESS_CYCLES[
    (bass.MemorySpace.PSUM, mybir.EngineType.DVE)] = 380

B, S, D, H = 2, 2048, 1024, 16
DK = D // H                  # 64 head dim
NCORE = 8
GROUPS = NCORE // B          # 4 head-groups per batch
HPC = H // GROUPS            # 4 heads per core
FPC = HPC * DK               # 256 features per core
FT = FPC // 128              # 2 feature tiles (= head pairs) per core
DT = D // 128                # 8 d_in tiles
TT = S // 128                # 16 token tiles (k tiles)
QB = 512                     # query block (free-dim) size in attention
NQB = S // QB                # 4 query blocks = pipeline stages
BF = mybir.dt.bfloat16
F32 = mybir.dt.float32
BFNP = ml_dtypes.bfloat16

# module-level knobs for test.py
PROFILE = False
TRACE_CORES = None
LAST_RESULT = None

_program_cache: dict = {}


def _classify_mask(mask2d: np.ndarray):
    """Classify (S, S) keep-mask into per-(qblock, ktile) modes.

    Returns (plan, patterns): plan[qb] is a list of (kt, mask_id|None, c0, c1)
    for tiles that are at least partially kept, where c0 is the first
    q-column (within the block) with any kept key and [c0, c1) the columns
    needing the multiplicative mask; patterns[mid] is a [128, 2, c1-c0] bf16
    multiplicative mask (k on partitions, duplicated across the head axis).
    """
    keep = np.asarray(mask2d) != 0
    patterns = []
    pattern_ids = {}
    plan = []
    for qb in range(NQB):
        row = []
        for kt in range(TT):
            blk = keep[qb * QB:(qb + 1) * QB, kt * 128:(kt + 1) * 128].T
            if not blk.any():
                continue
            if blk.all():
                row.append((kt, None, 0, 0))
                continue
            anyk = blk.any(axis=0)
            allk = blk.all(axis=0)
            c0 = int(np.flatnonzero(anyk)[0])
            notall = np.flatnonzero(~allk)
            c1 = int(notall[-1]) + 1 if notall.size else c0
            pat = blk[:, c0:c1]
            key = pat.tobytes()
            mid = pattern_ids.get(key)
            if mid is None:
                mid = len(patterns)
                pattern_ids[key] = mid
                # duplicate across the 2-head axis of the pT tile
                patterns.append(np.repeat(pat[:, None, :], 2, axis=1)
                                .astype(BFNP))
            row.append((kt, mid, c0, c1))
        plan.append(row)
    return plan, patterns


def build_program(plan, npat, pw):
    nc = bacc.Bacc("TRN2", target_bir_lowering=False, debug=False,
                   num_devices=NCORE)
    qT = nc.dram_tensor("qT", (D, S), BF, kind="ExternalInput").ap()
    kT = nc.dram_tensor("kT", (D, S), BF, kind="ExternalInput").ap()
    vT = nc.dram_tensor("vT", (D, S), BF, kind="ExternalInput").ap()
    wqT = nc.dram_tensor("wqT", (D, FPC), BF, kind="ExternalInput").ap()
    wkT = nc.dram_tensor("wkT", (D, FPC), BF, kind="ExternalInput").ap()
    wvT = nc.dram_tensor("wvT", (D, FPC), BF, kind="ExternalInput").ap()
    woT = nc.dram_tensor("woT", (FPC, D), BF, kind="ExternalInput").ap()
    bqk = nc.dram_tensor("bqk", (2, FT, 128), F32, kind="ExternalInput").ap()
    masks = None
    if npat:
        masks = nc.dram_tensor("masks", (npat, 128, 2 * pw), BF,
                               kind="ExternalInput").ap()
    out = nc.dram_tensor("out", (S, D), BF, kind="ExternalOutput").ap()

    with tile.TileContext(nc) as tc, ExitStack() as ctx:
        singles = ctx.enter_context(tc.tile_pool(name="singles", bufs=1))
        xpool = ctx.enter_context(tc.tile_pool(name="xpool", bufs=2))
        ppool = ctx.enter_context(tc.tile_pool(name="ppool", bufs=4))
        npool = ctx.enter_context(tc.tile_pool(name="npool", bufs=2))
        upool = ctx.enter_context(tc.tile_pool(name="upool", bufs=2))
        opool = ctx.enter_context(tc.tile_pool(name="opool", bufs=2))
        psS = ctx.enter_context(tc.tile_pool(name="psS", bufs=2, space="PSUM"))
        psPV = ctx.enter_context(tc.tile_pool(name="psPV", bufs=1, space="PSUM"))

        # ---- SBUF residents ----
        wq_sb = singles.tile([128, DT, FPC], BF)
        wk_sb = singles.tile([128, DT, FPC], BF)
        wv_sb = singles.tile([128, DT, FPC], BF)
        wo_sb = singles.tile([128, FT, D], BF)
        bias_sb = singles.tile([128, 2, FT], F32)
        mask_sb = None
        if npat:
            mask_sb = singles.tile([128, npat, 2 * pw], BF, name="mask_sb")
        q_sb = singles.tile([128, FT, S], BF)
        k_sb = singles.tile([128, FT, S], BF)
        attn_sb = singles.tile([128, FT, S], BF)
        v_sb = singles.tile([128, TT, HPC, DK + 1], BF)

        # ---- input DMA issue order (sync queue, FIFO = priority): the
        # first compute (K0/Q0 projection, then the exp stream) needs
        # wk+xk0+wq+xq0, so those go first ----
        xk_c = [xpool.tile([128, DT, QB], BF, name=f"xk{c}", tag="xk")
                for c in range(NQB)]
        xq_c = [xpool.tile([128, DT, QB], BF, name=f"xq{c}", tag="xq")
                for c in range(NQB)]
        xv_c = [xpool.tile([128, DT, QB], BF, name=f"xv{c}", tag="xv")
                for c in range(NQB)]

        def chunk_dma(x_sb, x_dram, c):
            nc.sync.dma_start(
                x_sb,
                x_dram[:, c * QB:(c + 1) * QB]
                .rearrange("(t p) f -> p t f", p=128))

        # issue order on the sync ring = first-need order: K0/Q0 projection
        # inputs first (they gate the exp stream), then V0, then the rest
        nc.sync.dma_start(bias_sb, bqk.rearrange("a b p -> p a b"))
        nc.sync.dma_start(wk_sb, wkT.rearrange("(t p) f -> p t f", p=128))
        chunk_dma(xk_c[0], kT, 0)
        nc.sync.dma_start(wq_sb, wqT.rearrange("(t p) f -> p t f", p=128))
        chunk_dma(xq_c[0], qT, 0)
        nc.sync.dma_start(wv_sb, wvT.rearrange("(t p) f -> p t f", p=128))
        chunk_dma(xv_c[0], vT, 0)
        if npat:
            nc.sync.dma_start(mask_sb, masks.rearrange("m p f -> p m f"))
        nc.sync.dma_start(wo_sb, woT.rearrange("(t p) f -> p t f", p=128))
        for c in range(1, NQB):
            chunk_dma(xk_c[c], kT, c)
            chunk_dma(xq_c[c], qT, c)
            chunk_dma(xv_c[c], vT, c)

        # trailing ones column of V: PV's last output partition (64 — still
        # 32-aligned for VectorE access) accumulates the softmax denominator
        for tt in range(TT):
            nc.vector.memset(v_sb[:, tt, :, DK:DK + 1], 1.0)

        # PE warm-up: ~6us of dummy matmuls (one accumulation group, so only
        # one PSUM ring slot is held) while the first inputs stream in, so the
        # HAM clock gate is at 8/8 when the real work starts (and doesn't
        # re-throttle before the first projection's inputs land).
        warm_sb = singles.tile([128, QB], BF)
        nc.vector.memset(warm_sb, 0.0)
        ps = psS.tile([128, QB], F32, tag="acc", name="warm_ps")
        for r in range(14):
            nc.tensor.matmul(ps, lhsT=warm_sb[:, 0:128], rhs=warm_sb,
                             start=(r == 0), stop=(r == 13))

        inv_sqrt_dk = float(1.0 / np.sqrt(DK))

        def o_proj(qb):
            # partial O-projection for query block qb (deferred one stage so
            # the PE never waits on qb's normalize chain)
            ob = opool.tile([128, QB // 128, D], BF, name="ob")
            for qt in range(QB // 128):
                for nch in range(D // QB):
                    ps = psS.tile([128, QB], F32, tag="acc", name="ps")
                    for hd in range(FT):
                        nc.tensor.matmul(
                            ps,
                            lhsT=attn_sb[:, hd,
                                         qb * QB + qt * 128:
                                         qb * QB + (qt + 1) * 128],
                            rhs=wo_sb[:, hd, nch * QB:(nch + 1) * QB],
                            start=(hd == 0), stop=(hd == FT - 1))
                    nc.vector.tensor_copy(ob[:, qt, nch * QB:(nch + 1) * QB],
                                          ps)
                if qt % 2 == 1:
                    # write out each half as soon as its evacs land
                    nc.sync.dma_start(
                        out[qb * QB + (qt - 1) * 128:
                            qb * QB + (qt + 1) * 128, :]
                        .rearrange("(t p) f -> p t f", p=128),
                        ob[:, qt - 1:qt + 1, :])

        def kq_proj(c, ft):
            # K^T / Q^T projection for chunk c, one feature tile
            # ([feat, tok] layout; the evac adds the bias from PSUM)
            for bi, x_sb, w_sb, y_sb in ((1, xk_c[c], wk_sb, k_sb),
                                         (0, xq_c[c], wq_sb, q_sb)):
                ps = psS.tile([128, QB], F32, tag="acc", name="ps")
                for dt in range(DT):
                    nc.tensor.matmul(
                        ps,
                        lhsT=w_sb[:, dt, ft * 128:(ft + 1) * 128],
                        rhs=x_sb[:, dt, :],
                        start=(dt == 0), stop=(dt == DT - 1))
                nc.vector.tensor_scalar_add(
                    y_sb[:, ft, c * QB:(c + 1) * QB], ps,
                    bias_sb[:, bi, ft:ft + 1])

        def v_proj(c):
            # V projection for chunk c ([tok, feat] layout)
            for tt in range(4 * c, 4 * c + 4):
                ps = psS.tile([128, FPC], F32, tag="acc", name="ps")
                for dt in range(DT):
                    nc.tensor.matmul(
                        ps,
                        lhsT=xv_c[c][:, dt, (tt - 4 * c) * 128:
                                     (tt - 4 * c + 1) * 128],
                        rhs=wv_sb[:, dt, :],
                        start=(dt == 0), stop=(dt == DT - 1))
                nc.vector.tensor_copy(
                    v_sb[:, tt, :, 0:DK],
                    ps.rearrange("p (h d) -> p h d", h=HPC))

        def attention(qb, ft):
            # causal attention for query block qb, head pair ft
            kts = plan[qb]
            hA, hB = 2 * ft, 2 * ft + 1
            pv = psPV.tile([DK + 1, 2, QB], F32, name="pv")
            n = len(kts)
            pts = []
            for i, (kt, mid, c0, c1) in enumerate(kts):
                # QK^T for both heads: row groups 0-1 / 2-3, concurrent
                s = psS.tile([128, 2, QB], F32, tag="qk", name="s")
                for j, pr in ((0, 0), (1, 64)):
                    nc.tensor.matmul(
                        s[:, j, c0:],
                        lhsT=k_sb[pr:pr + DK, ft,
                                  kt * 128:(kt + 1) * 128],
                        rhs=q_sb[pr:pr + DK, ft,
                                 qb * QB + c0:(qb + 1) * QB],
                        start=True, stop=True)
                pT = ppool.tile([128, 2, QB], BF, tag="pt", name="pT")
                nc.scalar.activation(pT[:, :, c0:], s[:, :, c0:],
                                     mybir.ActivationFunctionType.Exp,
                                     scale=inv_sqrt_dk)
                if mid is not None and c1 > c0:
                    assert mask_sb is not None
                    w = c1 - c0
                    nc.vector.tensor_mul(
                        pT[:, :, c0:c1], pT[:, :, c0:c1],
                        mask_sb[:, mid, 0:2 * w].rearrange(
                            "p (j w) -> p j w", j=2))
                pts.append((kt, c0, pT))
                # software pipeline: PV lags one k-tile behind QK/exp
                if i > 0:
                    pkt, pc0, ppT = pts[i - 1]
                    for j, h in ((0, hA), (1, hB)):
                        nc.tensor.matmul(
                            pv[:, j, pc0:], lhsT=v_sb[:, pkt, h, :],
                            rhs=ppT[:, j, pc0:],
                            start=(i == 1), stop=False)
            pkt, pc0, ppT = pts[n - 1]
            for j, h in ((0, hA), (1, hB)):
                nc.tensor.matmul(
                    pv[:, j, pc0:], lhsT=v_sb[:, pkt, h, :],
                    rhs=ppT[:, j, pc0:],
                    start=(n == 1), stop=True)

            # normalize per column half (shorter PSUM hold, and the next
            # consumer can start on the first half earlier):
            # attn^T[d, q] = attnU^T[d, q] / denom[q], denom = pv partition 0
            HB2 = QB // 2
            for lo in (0, HB2):
                # ScalarE moves the denominator to partition 0 (the only
                # engine that can shift partitions; custom DVE ops require
                # base partition 0), then DVE reciprocal + GpSimd broadcast
                den = npool.tile([1, 2, HB2], F32, tag="den", name="den")
                nc.scalar.copy(den, pv[DK:DK + 1, :, lo:lo + HB2])
                u = upool.tile([DK, 2, HB2], BF, tag="u", name="u")
                nc.vector.tensor_copy(u, pv[0:DK, :, lo:lo + HB2])
                rec = npool.tile([1, 2, HB2], F32, tag="rec", name="rec")
                nc.vector.reciprocal_approx_fast(rec, den)
                bc = npool.tile([DK, 2, HB2], F32, tag="bc", name="bc")
                nc.gpsimd.partition_broadcast(bc, rec)
                qcols = slice(qb * QB + lo, qb * QB + lo + HB2)
                nc.vector.tensor_mul(attn_sb[0:DK, ft, qcols],
                                     u[:, 0, :], bc[:, 0, :])
                nc.vector.tensor_mul(attn_sb[64:64 + DK, ft, qcols],
                                     u[:, 1, :], bc[:, 1, :])

        # Stage pipeline.  The attention stream (QK -> exp -> PV) is
        # high-priority: the scheduler fills its pipeline stalls with
        # projection / O-projection matmuls instead of stalling the PE
        # queue.  Attention blocks run biggest-block-first at the end —
        # (0, ft=1), which needs only chunk 0, runs LAST so qb3's PV /
        # normalize / O-projection hide under its exp stream and the
        # kernel tail is the smallest block's.
        for c in range(NQB):
            kq_proj(c, 0)
            v_proj(c)
            with tc.high_priority():
                attention(c, 0)
            if c > 0:
                o_proj(c - 1)
            kq_proj(c, 1)
            with tc.high_priority():
                attention(c, 1)
        o_proj(NQB - 1)

    nc.compile()
    return nc


def _get_program(mask2d: np.ndarray):
    key = hashlib.sha1(np.ascontiguousarray(mask2d).tobytes()).hexdigest()
    hit = _program_cache.get(key)
    if hit is not None:
        return hit
    plan, patterns = _classify_mask(mask2d)
    pw = max((p.shape[2] for p in patterns), default=0)
    nc = build_program(plan, len(patterns), pw)
    if patterns:
        pat = np.zeros((len(patterns), 128, 2 * pw), BFNP)
        for i, p in enumerate(patterns):
            w = p.shape[2]
            pat[i, :, :2 * w] = p.reshape(128, 2 * w)
    else:
        pat = None
    _program_cache[key] = (nc, pat)
    return nc, pat


def kernel(**inputs) -> np.ndarray:
    global LAST_RESULT
    query = np.asarray(inputs["query"], np.float32)
    key = np.asarray(inputs["key"], np.float32)
    value = np.asarray(inputs["value"], np.float32)
    mask = np.asarray(inputs["mask"])
    Wq = np.asarray(inputs["Wq"], np.float32)
    bq = np.asarray(inputs["bq"], np.float32)
    Wk = np.asarray(inputs["Wk"], np.float32)
    bk = np.asarray(inputs["bk"], np.float32)
    Wv = np.asarray(inputs["Wv"], np.float32)
    bv = np.asarray(inputs["bv"], np.float32)
    Wo = np.asarray(inputs["Wo"], np.float32)
    bo = np.asarray(inputs["bo"], np.float32)

    nc, pat = _get_program(mask.reshape(S, S))

    WqT, WkT, WvT, WoT = Wq.T, Wk.T, Wv.T, Wo.T
    xT = {
        t: [np.ascontiguousarray(x[b].T).astype(BFNP) for b in range(B)]
        for t, x in (("qT", query), ("kT", key), ("vT", value))
    }
    in_maps = []
    for c in range(NCORE):
        b, g = divmod(c, GROUPS)
        f0 = g * FPC
        m = {
            "qT": xT["qT"][b],
            "kT": xT["kT"][b],
            "vT": xT["vT"][b],
            "wqT": np.ascontiguousarray(WqT[:, f0:f0 + FPC]).astype(BFNP),
            "wkT": np.ascontiguousarray(WkT[:, f0:f0 + FPC]).astype(BFNP),
            "wvT": np.ascontiguousarray(WvT[:, f0:f0 + FPC]).astype(BFNP),
            "woT": np.ascontiguousarray(WoT[f0:f0 + FPC, :]).astype(BFNP),
            "bqk": np.stack([bq[f0:f0 + FPC].reshape(FT, 128),
                             bk[f0:f0 + FPC].reshape(FT, 128)]).astype(np.float32),
        }
        if pat is not None:
            m["masks"] = pat
        in_maps.append(m)

    res = run_bass_kernel_spmd(
        nc, in_maps, core_ids=list(range(NCORE)),
        trace=PROFILE,
        trace_cores=(TRACE_CORES if TRACE_CORES is not None
                     else (list(range(NCORE)) if PROFILE else None)),
    )
    LAST_RESULT = res

    host_bias = bo + bv @ WoT  # (D,) folded V/O biases, added once per batch
    out = np.empty((B, S, D), np.float32)
    for b in range(B):
        acc = res.results[b * GROUPS]["out"].astype(np.float32)
        for g in range(1, GROUPS):
            acc = acc + res.results[b * GROUPS + g]["out"].astype(np.float32)
        out[b] = acc + host_bias
    return out


# revision 29
# speedup vs baseline: 1.3490x; 1.0848x over previous
"""Multi-head attention (B=2, S=2048, D=1024, H=16) on 8 Trainium2 NeuronCores.

Sharding: core c -> (batch b = c//4, head-group g = c%4).  Each core computes
Q/K/V projections for its 4 heads (256 features), causal attention for those
heads over the full sequence, and a partial O-projection (its 256 attn
features x full Wo.T slice).  The host sums the 4 bf16 partial outputs per
batch and folds in the biases that commute with the reduction (bo, bv @ Wo.T).

Pipeline (per core): the sequence is processed in 4 chunks of 512 tokens.
Stage c projects K/Q/V for chunk c, then runs causal attention for query
block c (which only needs K/V chunks 0..c), then the partial O-projection
for that block.  This starts the ScalarE exp stream (the serial bottleneck)
~45us earlier than projecting everything upfront.

Engine assignment:
  TensorE   projections + QK^T + PV + O-proj.  QK^T contracts over dk=64, so
            the two heads of a feature tile (partitions 0-63 / 64-127) issue
            back-to-back and run concurrently in different PE row groups
            (tile_position auto-derived from base partitions).
  ScalarE   only exp (fused 1/sqrt(dk) scale), one ACTIVATE per k-tile
            covering both heads of the pair via a [128, 2, QB] PSUM tile.
  VectorE   PSUM evacuations (+ K/Q bias add), mask multiplies, reciprocal,
            normalize multiplies.
  GpSimd    partition-broadcast of 1/denom; output DMA (SWDGE) so the Sync
            queue stays dedicated to input streaming.
  V layout  [tok, feat+1] with a ones column: PV also accumulates the
            softmax denominator (scores are O(5): no max subtraction needed,
            f32 exp cannot overflow).
"""

import hashlib
from contextlib import ExitStack

import ml_dtypes
import numpy as np

import concourse.bass as bass
import concourse.tile as tile
from concourse import bacc, hw_specs, mybir
from concourse.bass_utils import run_bass_kernel_spmd

# Calibrate the Tile scheduler's cost model to measured slice durations
# (ACTIVATE ~= (N + 625)/1.2 ns on HW vs the default 172-cycle init; DVE
# PSUM evacs measure ~(N + 380)/0.96).  With the default values the list
# scheduler believes the exp stream runs ~20% faster than it does, packs
# attention back-to-back, and defers projection matmuls past whole
# attention blocks — leaving the PE queue unable to fill real exp stalls.
# (cost-model calibration disabled — see git history)
if False:
    hw_specs.TRN2Spec.ACCESS_CYCLES[
        (bass.MemorySpace.PSUM, mybir.EngineType.Activation)] = 625
    hw_specs.TRN2Spec.ACCESS_CYCLES[
        (bass.MemorySpace.PSUM, mybir.EngineType.DVE)] = 380

B, S, D, H = 2, 2048, 1024, 16
DK = D // H                  # 64 head dim
NCORE = 8
GROUPS = NCORE // B          # 4 head-groups per batch
HPC = H // GROUPS            # 4 heads per core
FPC = HPC * DK               # 256 features per core
FT = FPC // 128              # 2 feature tiles (= head pairs) per core
DT = D // 128                # 8 d_in tiles
TT = S // 128                # 16 token tiles (k tiles)
QB = 512                     # query block (free-dim) size in attention
NQB = S // QB                # 4 query blocks = pipeline stages
BF = mybir.dt.bfloat16
F32 = mybir.dt.float32
BFNP = ml_dtypes.bfloat16

# module-level knobs for test.py
PROFILE = False
TRACE_CORES = None
LAST_RESULT = None

_program_cache: dict = {}


def _classify_mask(mask2d: np.ndarray):
    """Classify (S, S) keep-mask into per-(qblock, ktile) modes.

    Returns (plan, patterns): plan[qb] is a list of (kt, mask_id|None, c0, c1)
    for tiles that are at least partially kept, where c0 is the first
    q-column (within the block) with any kept key and [c0, c1) the columns
    needing the multiplicative mask; patterns[mid] is a [128, 2, c1-c0] bf16
    multiplicative mask (k on partitions, duplicated across the head axis).
    """
    keep = np.asarray(mask2d) != 0
    patterns = []
    pattern_ids = {}
    plan = []
    for qb in range(NQB):
        row = []
        for kt in range(TT):
            blk = keep[qb * QB:(qb + 1) * QB, kt * 128:(kt + 1) * 128].T
            if not blk.any():
                continue
            if blk.all():
                row.append((kt, None, 0, 0))
                continue
            anyk = blk.any(axis=0)
            allk = blk.all(axis=0)
            c0 = int(np.flatnonzero(anyk)[0])
            notall = np.flatnonzero(~allk)
            c1 = int(notall[-1]) + 1 if notall.size else c0
            pat = blk[:, c0:c1]
            key = pat.tobytes()
            mid = pattern_ids.get(key)
            if mid is None:
                mid = len(patterns)
                pattern_ids[key] = mid
                # duplicate across the 2-head axis of the pT tile
                patterns.append(np.repeat(pat[:, None, :], 2, axis=1)
                                .astype(BFNP))
            row.append((kt, mid, c0, c1))
        plan.append(row)
    return plan, patterns


def build_program(plan, npat, pw):
    nc = bacc.Bacc("TRN2", target_bir_lowering=False, debug=False,
                   num_devices=NCORE)
    qT = nc.dram_tensor("qT", (D, S), BF, kind="ExternalInput").ap()
    kT = nc.dram_tensor("kT", (D, S), BF, kind="ExternalInput").ap()
    vT = nc.dram_tensor("vT", (D, S), BF, kind="ExternalInput").ap()
    wqT = nc.dram_tensor("wqT", (D, FPC), BF, kind="ExternalInput").ap()
    wkT = nc.dram_tensor("wkT", (D, FPC), BF, kind="ExternalInput").ap()
    wvT = nc.dram_tensor("wvT", (D, FPC), BF, kind="ExternalInput").ap()
    woT = nc.dram_tensor("woT", (FPC, D), BF, kind="ExternalInput").ap()
    bqk = nc.dram_tensor("bqk", (2, FT, 128), F32, kind="ExternalInput").ap()
    masks = None
    if npat:
        masks = nc.dram_tensor("masks", (npat, 128, 2 * pw), BF,
                               kind="ExternalInput").ap()
    out = nc.dram_tensor("out", (S, D), BF, kind="ExternalOutput").ap()

    with tile.TileContext(nc) as tc, ExitStack() as ctx:
        singles = ctx.enter_context(tc.tile_pool(name="singles", bufs=1))
        xpool = ctx.enter_context(tc.tile_pool(name="xpool", bufs=2))
        ppool = ctx.enter_context(tc.tile_pool(name="ppool", bufs=4))
        npool = ctx.enter_context(tc.tile_pool(name="npool", bufs=2))
        upool = ctx.enter_context(tc.tile_pool(name="upool", bufs=2))
        opool = ctx.enter_context(tc.tile_pool(name="opool", bufs=2))
        psS = ctx.enter_context(tc.tile_pool(name="psS", bufs=2, space="PSUM"))
        psPV = ctx.enter_context(tc.tile_pool(name="psPV", bufs=1, space="PSUM"))

        # ---- SBUF residents ----
        wq_sb = singles.tile([128, DT, FPC], BF)
        wk_sb = singles.tile([128, DT, FPC], BF)
        wv_sb = singles.tile([128, DT, FPC], BF)
        wo_sb = singles.tile([128, FT, D], BF)
        bias_sb = singles.tile([128, 2, FT], F32)
        mask_sb = None
        if npat:
            mask_sb = singles.tile([128, npat, 2 * pw], BF, name="mask_sb")
        q_sb = singles.tile([128, FT, S], BF)
        k_sb = singles.tile([128, FT, S], BF)
        attn_sb = singles.tile([128, FT, S], BF)
        v_sb = singles.tile([128, TT, HPC, DK + 1], BF)

        # ---- input DMA issue order (sync queue, FIFO = priority): the
        # first compute (K0/Q0 projection, then the exp stream) needs
        # wk+xk0+wq+xq0, so those go first ----
        xk_c = [xpool.tile([128, DT, QB], BF, name=f"xk{c}", tag="xk")
                for c in range(NQB)]
        xq_c = [xpool.tile([128, DT, QB], BF, name=f"xq{c}", tag="xq")
                for c in range(NQB)]
        xv_c = [xpool.tile([128, DT, QB], BF, name=f"xv{c}", tag="xv")
                for c in range(NQB)]

        def chunk_dma(x_sb, x_dram, c):
            nc.sync.dma_start(
                x_sb,
                x_dram[:, c * QB:(c + 1) * QB]
                .rearrange("(t p) f -> p t f", p=128))

        # issue order on the sync ring = first-need order: K0/Q0 projection
        # inputs first (they gate the exp stream), then V0, then the rest
        nc.sync.dma_start(bias_sb, bqk.rearrange("a b p -> p a b"))
        nc.sync.dma_start(wk_sb, wkT.rearrange("(t p) f -> p t f", p=128))
        chunk_dma(xk_c[0], kT, 0)
        nc.sync.dma_start(wq_sb, wqT.rearrange("(t p) f -> p t f", p=128))
        chunk_dma(xq_c[0], qT, 0)
        nc.sync.dma_start(wv_sb, wvT.rearrange("(t p) f -> p t f", p=128))
        chunk_dma(xv_c[0], vT, 0)
        if npat:
            nc.sync.dma_start(mask_sb, masks.rearrange("m p f -> p m f"))
        nc.sync.dma_start(wo_sb, woT.rearrange("(t p) f -> p t f", p=128))
        for c in range(1, NQB):
            chunk_dma(xk_c[c], kT, c)
            chunk_dma(xq_c[c], qT, c)
            chunk_dma(xv_c[c], vT, c)

        # trailing ones column of V: PV's last output partition (64 — still
        # 32-aligned for VectorE access) accumulates the softmax denominator
        for tt in range(TT):
            nc.vector.memset(v_sb[:, tt, :, DK:DK + 1], 1.0)

        # PE warm-up: ~6us of dummy matmuls (one accumulation group, so only
        # one PSUM ring slot is held) while the first inputs stream in, so the
        # HAM clock gate is at 8/8 when the real work starts (and doesn't
        # re-throttle before the first projection's inputs land).
        warm_sb = singles.tile([128, QB], BF)
        nc.vector.memset(warm_sb, 0.0)
        ps = psS.tile([128, QB], F32, tag="acc", name="warm_ps")
        for r in range(14):
            nc.tensor.matmul(ps, lhsT=warm_sb[:, 0:128], rhs=warm_sb,
                             start=(r == 0), stop=(r == 13))

        inv_sqrt_dk = float(1.0 / np.sqrt(DK))

        def o_proj(qb):
            # partial O-projection for query block qb (deferred one stage so
            # the PE never waits on qb's normalize chain)
            ob = opool.tile([128, QB // 128, D], BF, name="ob")
            for qt in range(QB // 128):
                for nch in range(D // QB):
                    ps = psS.tile([128, QB], F32, tag="acc", name="ps")
                    for hd in range(FT):
                        nc.tensor.matmul(
                            ps,
                            lhsT=attn_sb[:, hd,
                                         qb * QB + qt * 128:
                                         qb * QB + (qt + 1) * 128],
                            rhs=wo_sb[:, hd, nch * QB:(nch + 1) * QB],
                            start=(hd == 0), stop=(hd == FT - 1))
                    nc.vector.tensor_copy(ob[:, qt, nch * QB:(nch + 1) * QB],
                                          ps)
                if qt % 2 == 1:
                    # write out each half as soon as its evacs land
                    nc.sync.dma_start(
                        out[qb * QB + (qt - 1) * 128:
                            qb * QB + (qt + 1) * 128, :]
                        .rearrange("(t p) f -> p t f", p=128),
                        ob[:, qt - 1:qt + 1, :])

        def kq_proj(c, ft):
            # K^T / Q^T projection for chunk c, one feature tile
            # ([feat, tok] layout; the evac adds the bias from PSUM)
            for bi, x_sb, w_sb, y_sb in ((1, xk_c[c], wk_sb, k_sb),
                                         (0, xq_c[c], wq_sb, q_sb)):
                ps = psS.tile([128, QB], F32, tag="acc", name="ps")
                for dt in range(DT):
                    nc.tensor.matmul(
                        ps,
                        lhsT=w_sb[:, dt, ft * 128:(ft + 1) * 128],
                        rhs=x_sb[:, dt, :],
                        start=(dt == 0), stop=(dt == DT - 1))
                nc.vector.tensor_scalar_add(
                    y_sb[:, ft, c * QB:(c + 1) * QB], ps,
                    bias_sb[:, bi, ft:ft + 1])

        def v_proj(c):
            # V projection for chunk c ([tok, feat] layout)
            for tt in range(4 * c, 4 * c + 4):
                ps = psS.tile([128, FPC], F32, tag="acc", name="ps")
                for dt in range(DT):
                    nc.tensor.matmul(
                        ps,
                        lhsT=xv_c[c][:, dt, (tt - 4 * c) * 128:
                                     (tt - 4 * c + 1) * 128],
                        rhs=wv_sb[:, dt, :],
                        start=(dt == 0), stop=(dt == DT - 1))
                nc.vector.tensor_copy(
                    v_sb[:, tt, :, 0:DK],
                    ps.rearrange("p (h d) -> p h d", h=HPC))

        def attention(qb, ft):
            # causal attention for query block qb, head pair ft
            kts = plan[qb]
            hA, hB = 2 * ft, 2 * ft + 1
            pv = psPV.tile([DK + 1, 2, QB], F32, name="pv")
            n = len(kts)
            pts = []
            for i, (kt, mid, c0, c1) in enumerate(kts):
                # QK^T for both heads: row groups 0-1 / 2-3, concurrent
                s = psS.tile([128, 2, QB], F32, tag="qk", name="s")
                for j, pr in ((0, 0), (1, 64)):
                    nc.tensor.matmul(
                        s[:, j, c0:],
                        lhsT=k_sb[pr:pr + DK, ft,
                                  kt * 128:(kt + 1) * 128],
                        rhs=q_sb[pr:pr + DK, ft,
                                 qb * QB + c0:(qb + 1) * QB],
                        start=True, stop=True)
                pT = ppool.tile([128, 2, QB], BF, tag="pt", name="pT")
                nc.scalar.activation(pT[:, :, c0:], s[:, :, c0:],
                                     mybir.ActivationFunctionType.Exp,
                                     scale=inv_sqrt_dk)
                if mid is not None and c1 > c0:
                    assert mask_sb is not None
                    w = c1 - c0
                    nc.vector.tensor_mul(
                        pT[:, :, c0:c1], pT[:, :, c0:c1],
                        mask_sb[:, mid, 0:2 * w].rearrange(
                            "p (j w) -> p j w", j=2))
                pts.append((kt, c0, pT))
                # software pipeline: PV lags one k-tile behind QK/exp
                if i > 0:
                    pkt, pc0, ppT = pts[i - 1]
                    for j, h in ((0, hA), (1, hB)):
                        nc.tensor.matmul(
                            pv[:, j, pc0:], lhsT=v_sb[:, pkt, h, :],
                            rhs=ppT[:, j, pc0:],
                            start=(i == 1), stop=False)
            pkt, pc0, ppT = pts[n - 1]
            for j, h in ((0, hA), (1, hB)):
                nc.tensor.matmul(
                    pv[:, j, pc0:], lhsT=v_sb[:, pkt, h, :],
                    rhs=ppT[:, j, pc0:],
                    start=(n == 1), stop=True)

            # normalize per column half (shorter PSUM hold, and the next
            # consumer can start on the first half earlier):
            # attn^T[d, q] = attnU^T[d, q] / denom[q], denom = pv partition 0
            HB2 = QB // 2
            for lo in (0, HB2):
                # ScalarE moves the denominator to partition 0 (the only
                # engine that can shift partitions; custom DVE ops require
                # base partition 0), then DVE reciprocal + GpSimd broadcast
                den = npool.tile([1, 2, HB2], F32, tag="den", name="den")
                nc.scalar.copy(den, pv[DK:DK + 1, :, lo:lo + HB2])
                u = upool.tile([DK, 2, HB2], BF, tag="u", name="u")
                nc.vector.tensor_copy(u, pv[0:DK, :, lo:lo + HB2])
                rec = npool.tile([1, 2, HB2], F32, tag="rec", name="rec")
                nc.vector.reciprocal_approx_fast(rec, den)
                bc = npool.tile([DK, 2, HB2], F32, tag="bc", name="bc")
                nc.gpsimd.partition_broadcast(bc, rec)
                qcols = slice(qb * QB + lo, qb * QB + lo + HB2)
                nc.vector.tensor_mul(attn_sb[0:DK, ft, qcols],
                                     u[:, 0, :], bc[:, 0, :])
                nc.vector.tensor_mul(attn_sb[64:64 + DK, ft, qcols],
                                     u[:, 1, :], bc[:, 1, :])

        # Stage pipeline.  The attention stream (QK -> exp -> PV) is
        # high-priority: the scheduler fills its pipeline stalls with
        # projection / O-projection matmuls instead of stalling the PE
        # queue.  Attention blocks run biggest-block-first at the end —
        # (0, ft=1), which needs only chunk 0, runs LAST so qb3's PV /
        # normalize / O-projection hide under its exp stream and the
        # kernel tail is the smallest block's.
        for c in range(NQB):
            kq_proj(c, 0)
            v_proj(c)
            with tc.high_priority():
                attention(c, 0)
            if c > 0:
                o_proj(c - 1)
            kq_proj(c, 1)
            with tc.high_priority():
                attention(c, 1)
        o_proj(NQB - 1)

    nc.compile()
    return nc


def _get_program(mask2d: np.ndarray):
    key = hashlib.sha1(np.ascontiguousarray(mask2d).tobytes()).hexdigest()
    hit = _program_cache.get(key)
    if hit is not None:
        return hit
    plan, patterns = _classify_mask(mask2d)
    pw = max((p.shape[2] for p in patterns), default=0)
    nc = build_program(plan, len(patterns), pw)
    if patterns:
        pat = np.zeros((len(patterns), 128, 2 * pw), BFNP)
        for i, p in enumerate(patterns):
            w = p.shape[2]
            pat[i, :, :2 * w] = p.reshape(128, 2 * w)
    else:
        pat = None
    _program_cache[key] = (nc, pat)
    return nc, pat


def kernel(**inputs) -> np.ndarray:
    global LAST_RESULT
    query = np.asarray(inputs["query"], np.float32)
    key = np.asarray(inputs["key"], np.float32)
    value = np.asarray(inputs["value"], np.float32)
    mask = np.asarray(inputs["mask"])
    Wq = np.asarray(inputs["Wq"], np.float32)
    bq = np.asarray(inputs["bq"], np.float32)
    Wk = np.asarray(inputs["Wk"], np.float32)
    bk = np.asarray(inputs["bk"], np.float32)
    Wv = np.asarray(inputs["Wv"], np.float32)
    bv = np.asarray(inputs["bv"], np.float32)
    Wo = np.asarray(inputs["Wo"], np.float32)
    bo = np.asarray(inputs["bo"], np.float32)

    nc, pat = _get_program(mask.reshape(S, S))

    WqT, WkT, WvT, WoT = Wq.T, Wk.T, Wv.T, Wo.T
    xT = {
        t: [np.ascontiguousarray(x[b].T).astype(BFNP) for b in range(B)]
        for t, x in (("qT", query), ("kT", key), ("vT", value))
    }
    in_maps = []
    for c in range(NCORE):
        b, g = divmod(c, GROUPS)
        f0 = g * FPC
        m = {
            "qT": xT["qT"][b],
            "kT": xT["kT"][b],
            "vT": xT["vT"][b],
            "wqT": np.ascontiguousarray(WqT[:, f0:f0 + FPC]).astype(BFNP),
            "wkT": np.ascontiguousarray(WkT[:, f0:f0 + FPC]).astype(BFNP),
            "wvT": np.ascontiguousarray(WvT[:, f0:f0 + FPC]).astype(BFNP),
            "woT": np.ascontiguousarray(WoT[f0:f0 + FPC, :]).astype(BFNP),
            "bqk": np.stack([bq[f0:f0 + FPC].reshape(FT, 128),
                             bk[f0:f0 + FPC].reshape(FT, 128)]).astype(np.float32),
        }
        if pat is not None:
            m["masks"] = pat
        in_maps.append(m)

    res = run_bass_kernel_spmd(
        nc, in_maps, core_ids=list(range(NCORE)),
        trace=PROFILE,
        trace_cores=(TRACE_CORES if TRACE_CORES is not None
                     else (list(range(NCORE)) if PROFILE else None)),
    )
    LAST_RESULT = res

    host_bias = bo + bv @ WoT  # (D,) folded V/O biases, added once per batch
    out = np.empty((B, S, D), np.float32)
    for b in range(B):
        acc = res.results[b * GROUPS]["out"].astype(np.float32)
        for g in range(1, GROUPS):
            acc = acc + res.results[b * GROUPS + g]["out"].astype(np.float32)
        out[b] = acc + host_bias
    return out


# revision 30
# speedup vs baseline: 1.3674x; 1.0136x over previous
"""Multi-head attention (B=2, S=2048, D=1024, H=16) on 8 Trainium2 NeuronCores.

Sharding: core c -> (batch b = c//4, head-group g = c%4).  Each core computes
Q/K/V projections for its 4 heads (256 features), causal attention for those
heads over the full sequence, and a partial O-projection (its 256 attn
features x full Wo.T slice).  The host sums the 4 bf16 partial outputs per
batch and folds in the biases that commute with the reduction (bo, bv @ Wo.T).

Pipeline (per core): the sequence is processed in 4 chunks of 512 tokens.
Stage c projects K/Q/V for chunk c, then runs causal attention for query
block c (which only needs K/V chunks 0..c), then the partial O-projection
for that block.  This starts the ScalarE exp stream (the serial bottleneck)
~45us earlier than projecting everything upfront.

Engine assignment:
  TensorE   projections + QK^T + PV + O-proj.  QK^T contracts over dk=64, so
            the two heads of a feature tile (partitions 0-63 / 64-127) issue
            back-to-back and run concurrently in different PE row groups
            (tile_position auto-derived from base partitions).
  ScalarE   only exp (fused 1/sqrt(dk) scale), one ACTIVATE per k-tile
            covering both heads of the pair via a [128, 2, QB] PSUM tile.
  VectorE   PSUM evacuations (+ K/Q bias add), mask multiplies, reciprocal,
            normalize multiplies.
  GpSimd    partition-broadcast of 1/denom; output DMA (SWDGE) so the Sync
            queue stays dedicated to input streaming.
  V layout  [tok, feat+1] with a ones column: PV also accumulates the
            softmax denominator (scores are O(5): no max subtraction needed,
            f32 exp cannot overflow).
"""

import hashlib
from contextlib import ExitStack

import ml_dtypes
import numpy as np

import concourse.bass as bass
import concourse.tile as tile
from concourse import bacc, hw_specs, mybir
from concourse.bass_utils import run_bass_kernel_spmd

# Calibrate the Tile scheduler's cost model to measured slice durations
# (ACTIVATE ~= (N + 625)/1.2 ns on HW vs the default 172-cycle init; DVE
# PSUM evacs measure ~(N + 380)/0.96).  With the default values the list
# scheduler believes the exp stream runs ~20% faster than it does, packs
# attention back-to-back, and defers projection matmuls past whole
# attention blocks — leaving the PE queue unable to fill real exp stalls.
# (cost-model calibration disabled — see git history)
if False:
    hw_specs.TRN2Spec.ACCESS_CYCLES[
        (bass.MemorySpace.PSUM, mybir.EngineType.Activation)] = 625
    hw_specs.TRN2Spec.ACCESS_CYCLES[
        (bass.MemorySpace.PSUM, mybir.EngineType.DVE)] = 380

B, S, D, H = 2, 2048, 1024, 16
DK = D // H                  # 64 head dim
NCORE = 8
GROUPS = NCORE // B          # 4 head-groups per batch
HPC = H // GROUPS            # 4 heads per core
FPC = HPC * DK               # 256 features per core
FT = FPC // 128              # 2 feature tiles (= head pairs) per core
DT = D // 128                # 8 d_in tiles
TT = S // 128                # 16 token tiles (k tiles)
QB = 512                     # query block (free-dim) size in attention
NQB = S // QB                # 4 query blocks = pipeline stages
BF = mybir.dt.bfloat16
F32 = mybir.dt.float32
BFNP = ml_dtypes.bfloat16

# module-level knobs for test.py
PROFILE = False
TRACE_CORES = None
LAST_RESULT = None

_program_cache: dict = {}


def _classify_mask(mask2d: np.ndarray):
    """Classify (S, S) keep-mask into per-(qblock, ktile) modes.

    Returns (plan, patterns): plan[qb] is a list of (kt, mask_id|None, c0, c1)
    for tiles that are at least partially kept, where c0 is the first
    q-column (within the block) with any kept key and [c0, c1) the columns
    needing the multiplicative mask; patterns[mid] is a [128, 2, c1-c0] bf16
    multiplicative mask (k on partitions, duplicated across the head axis).
    """
    keep = np.asarray(mask2d) != 0
    patterns = []
    pattern_ids = {}
    plan = []
    for qb in range(NQB):
        row = []
        for kt in range(TT):
            blk = keep[qb * QB:(qb + 1) * QB, kt * 128:(kt + 1) * 128].T
            if not blk.any():
                continue
            if blk.all():
                row.append((kt, None, 0, 0))
                continue
            anyk = blk.any(axis=0)
            allk = blk.all(axis=0)
            c0 = int(np.flatnonzero(anyk)[0])
            notall = np.flatnonzero(~allk)
            c1 = int(notall[-1]) + 1 if notall.size else c0
            pat = blk[:, c0:c1]
            key = pat.tobytes()
            mid = pattern_ids.get(key)
            if mid is None:
                mid = len(patterns)
                pattern_ids[key] = mid
                # duplicate across the 2-head axis of the pT tile
                patterns.append(np.repeat(pat[:, None, :], 2, axis=1)
                                .astype(BFNP))
            row.append((kt, mid, c0, c1))
        plan.append(row)
    return plan, patterns


def build_program(plan, npat, pw):
    nc = bacc.Bacc("TRN2", target_bir_lowering=False, debug=False,
                   num_devices=NCORE)
    qT = nc.dram_tensor("qT", (D, S), BF, kind="ExternalInput").ap()
    kT = nc.dram_tensor("kT", (D, S), BF, kind="ExternalInput").ap()
    vT = nc.dram_tensor("vT", (D, S), BF, kind="ExternalInput").ap()
    wqT = nc.dram_tensor("wqT", (D, FPC), BF, kind="ExternalInput").ap()
    wkT = nc.dram_tensor("wkT", (D, FPC), BF, kind="ExternalInput").ap()
    wvT = nc.dram_tensor("wvT", (D, FPC), BF, kind="ExternalInput").ap()
    woT = nc.dram_tensor("woT", (FPC, D), BF, kind="ExternalInput").ap()
    bqk = nc.dram_tensor("bqk", (2, FT, 128), F32, kind="ExternalInput").ap()
    masks = None
    if npat:
        masks = nc.dram_tensor("masks", (npat, 128, 2 * pw), BF,
                               kind="ExternalInput").ap()
    out = nc.dram_tensor("out", (S, D), BF, kind="ExternalOutput").ap()

    with tile.TileContext(nc) as tc, ExitStack() as ctx:
        singles = ctx.enter_context(tc.tile_pool(name="singles", bufs=1))
        xpool = ctx.enter_context(tc.tile_pool(name="xpool", bufs=2))
        ppool = ctx.enter_context(tc.tile_pool(name="ppool", bufs=4))
        npool = ctx.enter_context(tc.tile_pool(name="npool", bufs=2))
        upool = ctx.enter_context(tc.tile_pool(name="upool", bufs=2))
        opool = ctx.enter_context(tc.tile_pool(name="opool", bufs=2))
        psS = ctx.enter_context(tc.tile_pool(name="psS", bufs=2, space="PSUM"))
        psPV = ctx.enter_context(tc.tile_pool(name="psPV", bufs=1, space="PSUM"))

        # ---- SBUF residents ----
        wq_sb = singles.tile([128, DT, FPC], BF)
        wk_sb = singles.tile([128, DT, FPC], BF)
        wv_sb = singles.tile([128, DT, FPC], BF)
        wo_sb = singles.tile([128, FT, D], BF)
        bias_sb = singles.tile([128, 2, FT], F32)
        mask_sb = None
        if npat:
            mask_sb = singles.tile([128, npat, 2 * pw], BF, name="mask_sb")
        q_sb = singles.tile([128, FT, S], BF)
        k_sb = singles.tile([128, FT, S], BF)
        attn_sb = singles.tile([128, FT, S], BF)
        v_sb = singles.tile([128, TT, HPC, DK + 1], BF)

        # ---- input DMA issue order (sync queue, FIFO = priority): the
        # first compute (K0/Q0 projection, then the exp stream) needs
        # wk+xk0+wq+xq0, so those go first ----
        xk_c = [xpool.tile([128, DT, QB], BF, name=f"xk{c}", tag="xk")
                for c in range(NQB)]
        xq_c = [xpool.tile([128, DT, QB], BF, name=f"xq{c}", tag="xq")
                for c in range(NQB)]
        xv_c = [xpool.tile([128, DT, QB], BF, name=f"xv{c}", tag="xv")
                for c in range(NQB)]

        def chunk_dma(x_sb, x_dram, c):
            nc.sync.dma_start(
                x_sb,
                x_dram[:, c * QB:(c + 1) * QB]
                .rearrange("(t p) f -> p t f", p=128))

        # issue order on the sync ring = first-need order: K0/Q0 projection
        # inputs first (they gate the exp stream), then V0, then the rest
        nc.sync.dma_start(bias_sb, bqk.rearrange("a b p -> p a b"))
        nc.sync.dma_start(wk_sb, wkT.rearrange("(t p) f -> p t f", p=128))
        chunk_dma(xk_c[0], kT, 0)
        nc.sync.dma_start(wq_sb, wqT.rearrange("(t p) f -> p t f", p=128))
        chunk_dma(xq_c[0], qT, 0)
        nc.sync.dma_start(wv_sb, wvT.rearrange("(t p) f -> p t f", p=128))
        chunk_dma(xv_c[0], vT, 0)
        if npat:
            nc.sync.dma_start(mask_sb, masks.rearrange("m p f -> p m f"))
        nc.sync.dma_start(wo_sb, woT.rearrange("(t p) f -> p t f", p=128))
        for c in range(1, NQB):
            chunk_dma(xk_c[c], kT, c)
            chunk_dma(xq_c[c], qT, c)
            chunk_dma(xv_c[c], vT, c)

        # trailing ones column of V: PV's last output partition (64 — still
        # 32-aligned for VectorE access) accumulates the softmax denominator
        for tt in range(TT):
            nc.vector.memset(v_sb[:, tt, :, DK:DK + 1], 1.0)

        # PE warm-up: ~6us of dummy matmuls (one accumulation group, so only
        # one PSUM ring slot is held) while the first inputs stream in, so the
        # HAM clock gate is at 8/8 when the real work starts (and doesn't
        # re-throttle before the first projection's inputs land).
        warm_sb = singles.tile([128, QB], BF)
        nc.vector.memset(warm_sb, 0.0)
        ps = psS.tile([128, QB], F32, tag="acc", name="warm_ps")
        for r in range(14):
            nc.tensor.matmul(ps, lhsT=warm_sb[:, 0:128], rhs=warm_sb,
                             start=(r == 0), stop=(r == 13))

        inv_sqrt_dk = float(1.0 / np.sqrt(DK))

        def o_proj(qb):
            # partial O-projection for query block qb (deferred one stage so
            # the PE never waits on qb's normalize chain)
            ob = opool.tile([128, QB // 128, D], BF, name="ob")
            for qt in range(QB // 128):
                for nch in range(D // QB):
                    ps = psS.tile([128, QB], F32, tag="acc", name="ps")
                    for hd in range(FT):
                        nc.tensor.matmul(
                            ps,
                            lhsT=attn_sb[:, hd,
                                         qb * QB + qt * 128:
                                         qb * QB + (qt + 1) * 128],
                            rhs=wo_sb[:, hd, nch * QB:(nch + 1) * QB],
                            start=(hd == 0), stop=(hd == FT - 1))
                    nc.vector.tensor_copy(ob[:, qt, nch * QB:(nch + 1) * QB],
                                          ps)
                if qt % 2 == 1:
                    # write out each half as soon as its evacs land
                    nc.sync.dma_start(
                        out[qb * QB + (qt - 1) * 128:
                            qb * QB + (qt + 1) * 128, :]
                        .rearrange("(t p) f -> p t f", p=128),
                        ob[:, qt - 1:qt + 1, :])

        def kq_proj(c, ft):
            # K^T / Q^T projection for chunk c, one feature tile
            # ([feat, tok] layout; the evac adds the bias from PSUM)
            for bi, x_sb, w_sb, y_sb in ((1, xk_c[c], wk_sb, k_sb),
                                         (0, xq_c[c], wq_sb, q_sb)):
                ps = psS.tile([128, QB], F32, tag="acc", name="ps")
                for dt in range(DT):
                    nc.tensor.matmul(
                        ps,
                        lhsT=w_sb[:, dt, ft * 128:(ft + 1) * 128],
                        rhs=x_sb[:, dt, :],
                        start=(dt == 0), stop=(dt == DT - 1))
                nc.vector.tensor_scalar_add(
                    y_sb[:, ft, c * QB:(c + 1) * QB], ps,
                    bias_sb[:, bi, ft:ft + 1])

        def v_proj(c):
            # V projection for chunk c ([tok, feat] layout)
            for tt in range(4 * c, 4 * c + 4):
                ps = psS.tile([128, FPC], F32, tag="acc", name="ps")
                for dt in range(DT):
                    nc.tensor.matmul(
                        ps,
                        lhsT=xv_c[c][:, dt, (tt - 4 * c) * 128:
                                     (tt - 4 * c + 1) * 128],
                        rhs=wv_sb[:, dt, :],
                        start=(dt == 0), stop=(dt == DT - 1))
                nc.vector.tensor_copy(
                    v_sb[:, tt, :, 0:DK],
                    ps.rearrange("p (h d) -> p h d", h=HPC))

        def attention(qb, ft):
            # causal attention for query block qb, head pair ft
            kts = plan[qb]
            hA, hB = 2 * ft, 2 * ft + 1
            pv = psPV.tile([DK + 1, 2, QB], F32, name="pv")
            n = len(kts)
            pts = []
            for i, (kt, mid, c0, c1) in enumerate(kts):
                # QK^T for both heads: row groups 0-1 / 2-3, concurrent
                s = psS.tile([128, 2, QB], F32, tag="qk", name="s")
                for j, pr in ((0, 0), (1, 64)):
                    nc.tensor.matmul(
                        s[:, j, c0:],
                        lhsT=k_sb[pr:pr + DK, ft,
                                  kt * 128:(kt + 1) * 128],
                        rhs=q_sb[pr:pr + DK, ft,
                                 qb * QB + c0:(qb + 1) * QB],
                        start=True, stop=True)
                pT = ppool.tile([128, 2, QB], BF, tag="pt", name="pT")
                nc.scalar.activation(pT[:, :, c0:], s[:, :, c0:],
                                     mybir.ActivationFunctionType.Exp,
                                     scale=inv_sqrt_dk)
                if mid is not None and c1 > c0:
                    assert mask_sb is not None
                    w = c1 - c0
                    nc.vector.tensor_mul(
                        pT[:, :, c0:c1], pT[:, :, c0:c1],
                        mask_sb[:, mid, 0:2 * w].rearrange(
                            "p (j w) -> p j w", j=2))
                pts.append((kt, c0, pT))
                # software pipeline: PV lags one k-tile behind QK/exp
                if i > 0:
                    pkt, pc0, ppT = pts[i - 1]
                    for j, h in ((0, hA), (1, hB)):
                        nc.tensor.matmul(
                            pv[:, j, pc0:], lhsT=v_sb[:, pkt, h, :],
                            rhs=ppT[:, j, pc0:],
                            start=(i == 1), stop=False)
            pkt, pc0, ppT = pts[n - 1]
            for j, h in ((0, hA), (1, hB)):
                nc.tensor.matmul(
                    pv[:, j, pc0:], lhsT=v_sb[:, pkt, h, :],
                    rhs=ppT[:, j, pc0:],
                    start=(n == 1), stop=True)

            # normalize per column half (shorter PSUM hold, and the next
            # consumer can start on the first half earlier):
            # attn^T[d, q] = attnU^T[d, q] / denom[q], denom = pv partition 0
            HB2 = QB // 2
            for lo in (0, HB2):
                # ScalarE moves the denominator to partition 0 (the only
                # engine that can shift partitions; custom DVE ops require
                # base partition 0), then DVE reciprocal + GpSimd broadcast
                den = npool.tile([1, 2, HB2], F32, tag="den", name="den")
                nc.scalar.copy(den, pv[DK:DK + 1, :, lo:lo + HB2])
                u = upool.tile([DK, 2, HB2], BF, tag="u", name="u")
                nc.vector.tensor_copy(u, pv[0:DK, :, lo:lo + HB2])
                rec = npool.tile([1, 2, HB2], F32, tag="rec", name="rec")
                nc.vector.reciprocal_approx_fast(rec, den)
                bc = npool.tile([DK, 2, HB2], F32, tag="bc", name="bc")
                nc.gpsimd.partition_broadcast(bc, rec)
                qcols = slice(qb * QB + lo, qb * QB + lo + HB2)
                nc.vector.tensor_mul(attn_sb[0:DK, ft, qcols],
                                     u[:, 0, :], bc[:, 0, :])
                nc.vector.tensor_mul(attn_sb[64:64 + DK, ft, qcols],
                                     u[:, 1, :], bc[:, 1, :])

        # Stage pipeline.  The attention stream (QK -> exp -> PV) is
        # high-priority: the scheduler fills its pipeline stalls with
        # projection / O-projection matmuls instead of stalling the PE
        # queue.  Attention blocks run biggest-block-first at the end —
        # (0, ft=1), which needs only chunk 0, runs LAST so qb3's PV /
        # normalize / O-projection hide under its exp stream and the
        # kernel tail is the smallest block's.
        for c in range(NQB):
            kq_proj(c, 0)
            v_proj(c)
            attention(c, 0)
            if c > 0:
                o_proj(c - 1)
            kq_proj(c, 1)
            attention(c, 1)
        o_proj(NQB - 1)

    nc.compile()
    return nc


def _get_program(mask2d: np.ndarray):
    key = hashlib.sha1(np.ascontiguousarray(mask2d).tobytes()).hexdigest()
    hit = _program_cache.get(key)
    if hit is not None:
        return hit
    plan, patterns = _classify_mask(mask2d)
    pw = max((p.shape[2] for p in patterns), default=0)
    nc = build_program(plan, len(patterns), pw)
    if patterns:
        pat = np.zeros((len(patterns), 128, 2 * pw), BFNP)
        for i, p in enumerate(patterns):
            w = p.shape[2]
            pat[i, :, :2 * w] = p.reshape(128, 2 * w)
    else:
        pat = None
    _program_cache[key] = (nc, pat)
    return nc, pat


def kernel(**inputs) -> np.ndarray:
    global LAST_RESULT
    query = np.asarray(inputs["query"], np.float32)
    key = np.asarray(inputs["key"], np.float32)
    value = np.asarray(inputs["value"], np.float32)
    mask = np.asarray(inputs["mask"])
    Wq = np.asarray(inputs["Wq"], np.float32)
    bq = np.asarray(inputs["bq"], np.float32)
    Wk = np.asarray(inputs["Wk"], np.float32)
    bk = np.asarray(inputs["bk"], np.float32)
    Wv = np.asarray(inputs["Wv"], np.float32)
    bv = np.asarray(inputs["bv"], np.float32)
    Wo = np.asarray(inputs["Wo"], np.float32)
    bo = np.asarray(inputs["bo"], np.float32)

    nc, pat = _get_program(mask.reshape(S, S))

    WqT, WkT, WvT, WoT = Wq.T, Wk.T, Wv.T, Wo.T
    xT = {
        t: [np.ascontiguousarray(x[b].T).astype(BFNP) for b in range(B)]
        for t, x in (("qT", query), ("kT", key), ("vT", value))
    }
    in_maps = []
    for c in range(NCORE):
        b, g = divmod(c, GROUPS)
        f0 = g * FPC
        m = {
            "qT": xT["qT"][b],
            "kT": xT["kT"][b],
            "vT": xT["vT"][b],
            "wqT": np.ascontiguousarray(WqT[:, f0:f0 + FPC]).astype(BFNP),
            "wkT": np.ascontiguousarray(WkT[:, f0:f0 + FPC]).astype(BFNP),
            "wvT": np.ascontiguousarray(WvT[:, f0:f0 + FPC]).astype(BFNP),
            "woT": np.ascontiguousarray(WoT[f0:f0 + FPC, :]).astype(BFNP),
            "bqk": np.stack([bq[f0:f0 + FPC].reshape(FT, 128),
                             bk[f0:f0 + FPC].reshape(FT, 128)]).astype(np.float32),
        }
        if pat is not None:
            m["masks"] = pat
        in_maps.append(m)

    res = run_bass_kernel_spmd(
        nc, in_maps, core_ids=list(range(NCORE)),
        trace=PROFILE,
        trace_cores=(TRACE_CORES if TRACE_CORES is not None
                     else (list(range(NCORE)) if PROFILE else None)),
    )
    LAST_RESULT = res

    host_bias = bo + bv @ WoT  # (D,) folded V/O biases, added once per batch
    out = np.empty((B, S, D), np.float32)
    for b in range(B):
        acc = res.results[b * GROUPS]["out"].astype(np.float32)
        for g in range(1, GROUPS):
            acc = acc + res.results[b * GROUPS + g]["out"].astype(np.float32)
        out[b] = acc + host_bias
    return out


# revision 50
# speedup vs baseline: 1.4692x; 1.0745x over previous
"""Multi-head attention (B=2, S=2048, D=1024, H=16) on 8 Trainium2 NeuronCores.

Sharding: core c -> (batch b = c//4, head-group g = c%4).  Each core computes
Q/K/V projections for its 4 heads (256 features), causal attention for those
heads over the full sequence, and a partial O-projection (its 256 attn
features x full Wo.T slice).  The host sums the 4 bf16 partial outputs per
batch and folds in the biases that commute with the reduction (bo, bv @ Wo.T).

Pipeline (per core): the sequence is processed in 4 chunks of 512 tokens.
Stage c projects K/Q/V for chunk c, then runs causal attention for query
block c (which only needs K/V chunks 0..c), then the partial O-projection
for that block.  This starts the ScalarE exp stream (the serial bottleneck)
~45us earlier than projecting everything upfront.

Engine assignment:
  TensorE   projections + QK^T + PV + O-proj.  QK^T contracts over dk=64, so
            the two heads of a feature tile (partitions 0-63 / 64-127) issue
            back-to-back and run concurrently in different PE row groups
            (tile_position auto-derived from base partitions).
  ScalarE   only exp (fused 1/sqrt(dk) scale), one ACTIVATE per k-tile
            covering both heads of the pair via a [128, 2, QB] PSUM tile.
  VectorE   PSUM evacuations (+ K/Q bias add), mask multiplies, reciprocal,
            normalize multiplies.
  GpSimd    partition-broadcast of 1/denom; output DMA (SWDGE) so the Sync
            queue stays dedicated to input streaming.
  V layout  [tok, feat+1] with a ones column: PV also accumulates the
            softmax denominator (scores are O(5): no max subtraction needed,
            f32 exp cannot overflow).
"""

import hashlib
from contextlib import ExitStack

import ml_dtypes
import numpy as np

import concourse.bass as bass
import concourse.tile as tile
from concourse import bacc, hw_specs, mybir
from concourse.bass_utils import run_bass_kernel_spmd

# Calibrate the Tile scheduler's cost model to measured slice durations
# (ACTIVATE ~= (N + 625)/1.2 ns on HW vs the default 172-cycle init; DVE
# PSUM evacs measure ~(N + 380)/0.96).  With the default values the list
# scheduler believes the exp stream runs ~20% faster than it does, packs
# attention back-to-back, and defers projection matmuls past whole
# attention blocks — leaving the PE queue unable to fill real exp stalls.
# (cost-model calibration disabled — see git history)
if False:
    hw_specs.TRN2Spec.ACCESS_CYCLES[
        (bass.MemorySpace.PSUM, mybir.EngineType.Activation)] = 625
    hw_specs.TRN2Spec.ACCESS_CYCLES[
        (bass.MemorySpace.PSUM, mybir.EngineType.DVE)] = 380

B, S, D, H = 2, 2048, 1024, 16
DK = D // H                  # 64 head dim
NCORE = 8
GROUPS = NCORE // B          # 4 head-groups per batch
HPC = H // GROUPS            # 4 heads per core
FPC = HPC * DK               # 256 features per core
FT = FPC // 128              # 2 feature tiles (= head pairs) per core
DT = D // 128                # 8 d_in tiles
TT = S // 128                # 16 token tiles (k tiles)
QB = 512                     # query block (free-dim) size in attention
NQB = S // QB                # 4 query blocks = pipeline stages
BF = mybir.dt.bfloat16
F32 = mybir.dt.float32
BFNP = ml_dtypes.bfloat16

# module-level knobs for test.py
PROFILE = False
TRACE_CORES = None
LAST_RESULT = None

_program_cache: dict = {}


def _classify_mask(mask2d: np.ndarray):
    """Classify (S, S) keep-mask into per-(qblock, ktile) modes.

    Returns (plan, patterns): plan[qb] is a list of (kt, mask_id|None, c0, c1)
    for tiles that are at least partially kept, where c0 is the first
    q-column (within the block) with any kept key and [c0, c1) the columns
    needing the multiplicative mask; patterns[mid] is a [128, 2, c1-c0] bf16
    multiplicative mask (k on partitions, duplicated across the head axis).
    """
    keep = np.asarray(mask2d) != 0
    patterns = []
    pattern_ids = {}
    plan = []
    for qb in range(NQB):
        row = []
        for kt in range(TT):
            blk = keep[qb * QB:(qb + 1) * QB, kt * 128:(kt + 1) * 128].T
            if not blk.any():
                continue
            if blk.all():
                row.append((kt, None, 0, 0))
                continue
            anyk = blk.any(axis=0)
            allk = blk.all(axis=0)
            c0 = int(np.flatnonzero(anyk)[0])
            notall = np.flatnonzero(~allk)
            c1 = int(notall[-1]) + 1 if notall.size else c0
            pat = blk[:, c0:c1]
            key = pat.tobytes()
            mid = pattern_ids.get(key)
            if mid is None:
                mid = len(patterns)
                pattern_ids[key] = mid
                # duplicate across the 2-head axis of the pT tile
                patterns.append(np.repeat(pat[:, None, :], 2, axis=1)
                                .astype(BFNP))
            row.append((kt, mid, c0, c1))
        plan.append(row)
    return plan, patterns


def build_program(plan, npat, pw):
    nc = bacc.Bacc("TRN2", target_bir_lowering=False, debug=False,
                   num_devices=NCORE)
    qT = nc.dram_tensor("qT", (D, S), BF, kind="ExternalInput").ap()
    kT = nc.dram_tensor("kT", (D, S), BF, kind="ExternalInput").ap()
    vT = nc.dram_tensor("vT", (D, S), BF, kind="ExternalInput").ap()
    wqT = nc.dram_tensor("wqT", (D, FPC), BF, kind="ExternalInput").ap()
    wkT = nc.dram_tensor("wkT", (D, FPC), BF, kind="ExternalInput").ap()
    wvT = nc.dram_tensor("wvT", (D, FPC), BF, kind="ExternalInput").ap()
    woT = nc.dram_tensor("woT", (FPC, D), BF, kind="ExternalInput").ap()
    bqk = nc.dram_tensor("bqk", (2, FT, 128), F32, kind="ExternalInput").ap()
    masks = None
    if npat:
        masks = nc.dram_tensor("masks", (npat, 128, 2 * pw), BF,
                               kind="ExternalInput").ap()
    out = nc.dram_tensor("out", (S, D), BF, kind="ExternalOutput").ap()

    with tile.TileContext(nc) as tc, ExitStack() as ctx:
        singles = ctx.enter_context(tc.tile_pool(name="singles", bufs=1))
        xpool = ctx.enter_context(tc.tile_pool(name="xpool", bufs=2))
        ppool = ctx.enter_context(tc.tile_pool(name="ppool", bufs=6))
        npool = ctx.enter_context(tc.tile_pool(name="npool", bufs=2))
        upool = ctx.enter_context(tc.tile_pool(name="upool", bufs=2))
        opool = ctx.enter_context(tc.tile_pool(name="opool", bufs=2))
        psS = ctx.enter_context(tc.tile_pool(name="psS", bufs=2, space="PSUM"))
        psPV = ctx.enter_context(tc.tile_pool(name="psPV", bufs=1, space="PSUM"))

        # ---- SBUF residents ----
        wq_sb = singles.tile([128, DT, FPC], BF)
        wk_sb = singles.tile([128, DT, FPC], BF)
        wv_sb = singles.tile([128, DT, FPC], BF)
        wo_sb = singles.tile([128, FT, D], BF)
        bias_sb = singles.tile([128, 2, FT], F32)
        mask_sb = None
        if npat:
            mask_sb = singles.tile([128, npat, 2 * pw], BF, name="mask_sb")
        q_sb = singles.tile([128, FT, S], BF)
        k_sb = singles.tile([128, FT, S], BF)
        attn_sb = singles.tile([128, FT, S], BF)
        v_sb = singles.tile([128, TT, HPC, DK + 1], BF)

        # ---- input DMA issue order (sync queue, FIFO = priority): the
        # first compute (K0/Q0 projection, then the exp stream) needs
        # wk+xk0+wq+xq0, so those go first ----
        xk_c = [xpool.tile([128, DT, QB], BF, name=f"xk{c}", tag="xk")
                for c in range(NQB)]
        xq_c = [xpool.tile([128, DT, QB], BF, name=f"xq{c}", tag="xq")
                for c in range(NQB)]
        xv_c = [xpool.tile([128, DT, QB], BF, name=f"xv{c}", tag="xv")
                for c in range(NQB)]

        def chunk_dma(x_sb, x_dram, c):
            nc.sync.dma_start(
                x_sb,
                x_dram[:, c * QB:(c + 1) * QB]
                .rearrange("(t p) f -> p t f", p=128))

        # issue order on the sync ring = first-need order: K0/Q0 projection
        # inputs first (they gate the exp stream), then V0, then the rest
        nc.sync.dma_start(bias_sb, bqk.rearrange("a b p -> p a b"))
        nc.sync.dma_start(wk_sb, wkT.rearrange("(t p) f -> p t f", p=128))
        chunk_dma(xk_c[0], kT, 0)
        nc.sync.dma_start(wq_sb, wqT.rearrange("(t p) f -> p t f", p=128))
        chunk_dma(xq_c[0], qT, 0)
        nc.sync.dma_start(wv_sb, wvT.rearrange("(t p) f -> p t f", p=128))
        chunk_dma(xv_c[0], vT, 0)
        if npat:
            nc.sync.dma_start(mask_sb, masks.rearrange("m p f -> p m f"))
        nc.sync.dma_start(wo_sb, woT.rearrange("(t p) f -> p t f", p=128))
        for c in range(1, NQB):
            chunk_dma(xk_c[c], kT, c)
            chunk_dma(xq_c[c], qT, c)
            chunk_dma(xv_c[c], vT, c)

        # trailing ones column of V: PV's last output partition (64 — still
        # 32-aligned for VectorE access) accumulates the softmax denominator
        for tt in range(TT):
            nc.vector.memset(v_sb[:, tt, :, DK:DK + 1], 1.0)

        # PE warm-up: ~6us of dummy matmuls (one accumulation group, so only
        # one PSUM ring slot is held) while the first inputs stream in, so the
        # HAM clock gate is at 8/8 when the real work starts (and doesn't
        # re-throttle before the first projection's inputs land).
        warm_sb = singles.tile([128, QB], BF)
        nc.vector.memset(warm_sb, 0.0)
        ps = psS.tile([128, QB], F32, tag="acc", name="warm_ps")
        for r in range(14):
            nc.tensor.matmul(ps, lhsT=warm_sb[:, 0:128], rhs=warm_sb,
                             start=(r == 0), stop=(r == 13))

        inv_sqrt_dk = float(1.0 / np.sqrt(DK))

        # ---- PE filler tasks: one PSUM accumulation group each, emitted
        # inside the attention loops to fill the PE while ScalarE streams
        # exp.  Cost estimates (ns) pace the interleave. ----

        def kq_task(c, ft, bi):
            # one K^T (bi=1) / Q^T (bi=0) projection group, split into two
            # 4-dt parts so pacing can interleave at sub-group granularity
            x_sb = xq_c[c] if bi == 0 else xk_c[c]
            w_sb = wq_sb if bi == 0 else wk_sb
            y_sb = q_sb if bi == 0 else k_sb
            box = {}

            def part1():
                box["ps"] = psS.tile([128, QB], F32, tag="acc", name="ps")
                for dt in range(4):
                    nc.tensor.matmul(
                        box["ps"],
                        lhsT=w_sb[:, dt, ft * 128:(ft + 1) * 128],
                        rhs=x_sb[:, dt, :],
                        start=(dt == 0), stop=False)

            def part2():
                for dt in range(4, DT):
                    nc.tensor.matmul(
                        box["ps"],
                        lhsT=w_sb[:, dt, ft * 128:(ft + 1) * 128],
                        rhs=x_sb[:, dt, :],
                        start=False, stop=(dt == DT - 1))
                nc.vector.tensor_scalar_add(
                    y_sb[:, ft, c * QB:(c + 1) * QB], box["ps"],
                    bias_sb[:, bi, ft:ft + 1])
            return [(950, part1), (950, part2)]

        def v_task(c, tt):
            # one V projection token-tile group, split into two 4-dt parts
            box = {}

            def part1():
                box["ps"] = psS.tile([128, FPC], F32, tag="acc", name="ps")
                for dt in range(4):
                    nc.tensor.matmul(
                        box["ps"],
                        lhsT=xv_c[c][:, dt, (tt - 4 * c) * 128:
                                     (tt - 4 * c + 1) * 128],
                        rhs=wv_sb[:, dt, :],
                        start=(dt == 0), stop=False)

            def part2():
                for dt in range(4, DT):
                    nc.tensor.matmul(
                        box["ps"],
                        lhsT=xv_c[c][:, dt, (tt - 4 * c) * 128:
                                     (tt - 4 * c + 1) * 128],
                        rhs=wv_sb[:, dt, :],
                        start=False, stop=(dt == DT - 1))
                nc.vector.tensor_copy(
                    v_sb[:, tt, :, 0:DK],
                    box["ps"].rearrange("p (h d) -> p h d", h=HPC))
            return [(550, part1), (550, part2)]

        def o_tasks(qb, tail=False):
            # partial O-projection for query block qb as 8 tasks (one per
            # (token-tile, column-half) PSUM group); each odd token tile
            # ends with its half-block output DMA.  Tail tasks evacuate on
            # the (then idle) ScalarE instead of the busy VectorE.
            obs = {}

            def mk(qt, nch):
                def emit():
                    if qt % 2 == 0 and nch == 0:
                        obs[qt // 2] = opool.tile([128, 2, D], BF, name="ob")
                    ob = obs[qt // 2]
                    ps = psS.tile([128, QB], F32, tag="acc", name="ps")
                    for hd in range(FT):
                        nc.tensor.matmul(
                            ps,
                            lhsT=attn_sb[:, hd,
                                         qb * QB + qt * 128:
                                         qb * QB + (qt + 1) * 128],
                            rhs=wo_sb[:, hd, nch * QB:(nch + 1) * QB],
                            start=(hd == 0), stop=(hd == FT - 1))
                    dst = ob[:, qt % 2, nch * QB:(nch + 1) * QB]
                    if tail:
                        nc.scalar.copy(dst, ps)
                    else:
                        nc.vector.tensor_copy(dst, ps)
                    if qt % 2 == 1 and nch == D // QB - 1:
                        nc.sync.dma_start(
                            out[qb * QB + (qt - 1) * 128:
                                qb * QB + (qt + 1) * 128, :]
                            .rearrange("(t p) f -> p t f", p=128),
                            ob)
                return (520, emit)
            return [mk(qt, nch) for qt in range(QB // 128)
                    for nch in range(D // QB)]

        tasks = []  # FIFO of [cost_ns, emit_fn, key]

        def drain_tasks(*keys):
            """Force-emit queued tasks (all, or those matching keys)."""
            rest = []
            for t in tasks:
                if not keys or t[2] in keys:
                    t[1]()
                else:
                    rest.append(t)
            tasks[:] = rest

        def qk_exp(qb, ft, kt, mid, c0, c1):
            """QK^T (both heads, concurrent row groups) + exp + mask."""
            s = psS.tile([128, 2, QB], F32, tag="qk", name="s")
            for j2, pr in ((0, 0), (1, 64)):
                nc.tensor.matmul(
                    s[:, j2, c0:],
                    lhsT=k_sb[pr:pr + DK, ft, kt * 128:(kt + 1) * 128],
                    rhs=q_sb[pr:pr + DK, ft, qb * QB + c0:(qb + 1) * QB],
                    start=True, stop=True)
            pT = ppool.tile([128, 2, QB], BF, tag="pt", name="pT")
            nc.scalar.activation(pT[:, :, c0:], s[:, :, c0:],
                                 mybir.ActivationFunctionType.Exp,
                                 scale=inv_sqrt_dk)
            if mid is not None and c1 > c0:
                assert mask_sb is not None
                w = c1 - c0
                nc.vector.tensor_mul(
                    pT[:, :, c0:c1], pT[:, :, c0:c1],
                    mask_sb[:, mid, 0:2 * w].rearrange(
                        "p (j w) -> p j w", j=2))
            return pT

        def normalize(qb, ft, pv):
            """attn^T[d, q] = attnU^T[d, q] / denom[q], per column half."""
            HB2 = QB // 2
            for lo in (0, HB2):
                den = npool.tile([1, 2, HB2], F32, tag="den", name="den")
                nc.scalar.copy(den, pv[DK:DK + 1, :, lo:lo + HB2])
                u = upool.tile([DK, 2, HB2], BF, tag="u", name="u")
                nc.vector.tensor_copy(u, pv[0:DK, :, lo:lo + HB2])
                rec = npool.tile([1, 2, HB2], F32, tag="rec", name="rec")
                nc.vector.reciprocal_approx_fast(rec, den)
                bc = npool.tile([DK, 2, HB2], F32, tag="bc", name="bc")
                nc.gpsimd.partition_broadcast(bc, rec)
                qcols = slice(qb * QB + lo, qb * QB + lo + HB2)
                nc.vector.tensor_mul(attn_sb[0:DK, ft, qcols],
                                     u[:, 0, :], bc[:, 0, :])
                nc.vector.tensor_mul(attn_sb[64:64 + DK, ft, qcols],
                                     u[:, 1, :], bc[:, 1, :])

        # ---- merged global pipeline over all (block, k-tile) steps ----
        # The PV stream lags the QK/exp stream by one step ACROSS block
        # boundaries, so the serial ScalarE exp stream never stalls at a
        # pair boundary.  Q/K/V projection groups carry global-step
        # deadlines (Q before its block; K/V just before the k-tiles that
        # need them); O-projections pace into whatever slack remains.
        def enq(key, ts):
            tasks.extend([cost, emit, key] for cost, emit in ts)

        blocks = [(qb, ft) for qb in range(NQB) for ft in range(FT)]
        sched = []
        starts = {}
        for qb, ft in blocks:
            starts[(qb, ft)] = len(sched)
            for i2, rec in enumerate(plan[qb]):
                sched.append((qb, ft, i2, rec, len(plan[qb])))
        nsteps = len(sched)

        from collections import defaultdict
        due_map = defaultdict(list)
        enq_map = defaultdict(list)
        for i2 in range(4):
            for p, t in enumerate(v_task(0, i2)):
                due_map[i2 + p].append(t)
        for bi in (0, 1):
            for p, t in enumerate(kq_task(0, 1, bi)):
                due_map[2 + p].append(t)
        for c in range(1, NQB):
            for ft in range(FT):
                g0 = starts[(c, ft)]
                for p, t in enumerate(kq_task(c, ft, 0)):
                    due_map[max(0, g0 - 4 + p)].append(t)
                for p, t in enumerate(kq_task(c, ft, 1)):
                    due_map[g0 + 4 * c - 3 + p].append(t)
            g0 = starts[(c, 0)]
            for j2 in range(4):
                for p, t in enumerate(v_task(c, 4 * c + j2)):
                    due_map[g0 + 4 * c + j2 - 2 + p].append(t)
            # o(c-1) becomes available once pair (c-1, 1) normalizes,
            # which happens at step starts[(c, 0)] + 1
            enq_map[starts[(c, 0)] + 2].append((f"o{c - 1}",
                                                o_tasks(c - 1)))

        for _, part in kq_task(0, 0, 1) + kq_task(0, 0, 0):
            part()

        pv = None
        prev = None
        credit, debt = 0.0, 0.0
        for g in range(nsteps):
            qb, ft, i2, (kt, mid, c0, c1), n = sched[g]
            if i2 == 0:
                pv = psPV.tile([DK + 1, 2, QB], F32, name="pv")
            pT = qk_exp(qb, ft, kt, mid, c0, c1)
            debt += (QB - c0) / 2.4 + 90
            credit += (2 * (QB - c0) + 520) / 1.2
            for t in due_map.get(g, ()):
                t[1]()
                debt += t[0]
            for key, ts in enq_map.get(g, ()):
                enq(key, ts)
            # forget deficits older than a few microseconds: pacing should
            # track the CURRENT slack, not the startup's PE-bound backlog
            debt = max(debt, credit - 4000)
            credit = max(credit, debt - 4000)
            while tasks and debt + tasks[0][0] <= credit + 500:
                t = tasks.pop(0)
                t[1]()
                debt += t[0]
            if prev is not None:
                pqb, pft, pi, pkt, pc0, pn, ppT, ppv = prev
                for j2, h in ((0, 2 * pft), (1, 2 * pft + 1)):
                    nc.tensor.matmul(
                        ppv[:, j2, pc0:], lhsT=v_sb[:, pkt, h, :],
                        rhs=ppT[:, j2, pc0:],
                        start=(pi == 0), stop=(pi == pn - 1))
                debt += 2 * ((QB - pc0) / 2.4 + 50)
                if pi == pn - 1:
                    normalize(pqb, pft, ppv)
            prev = (qb, ft, i2, kt, c0, n, pT, pv)
        pqb, pft, pi, pkt, pc0, pn, ppT, ppv = prev
        for j2, h in ((0, 2 * pft), (1, 2 * pft + 1)):
            nc.tensor.matmul(
                ppv[:, j2, pc0:], lhsT=v_sb[:, pkt, h, :],
                rhs=ppT[:, j2, pc0:],
                start=(pi == 0), stop=(pi == pn - 1))
        normalize(pqb, pft, ppv)
        drain_tasks()
        for cost, emit in o_tasks(NQB - 1, tail=True):
            emit()

    nc.compile()
    return nc


def _get_program(mask2d: np.ndarray):
    key = hashlib.sha1(np.ascontiguousarray(mask2d).tobytes()).hexdigest()
    hit = _program_cache.get(key)
    if hit is not None:
        return hit
    plan, patterns = _classify_mask(mask2d)
    pw = max((p.shape[2] for p in patterns), default=0)
    nc = build_program(plan, len(patterns), pw)
    if patterns:
        pat = np.zeros((len(patterns), 128, 2 * pw), BFNP)
        for i, p in enumerate(patterns):
            w = p.shape[2]
            pat[i, :, :2 * w] = p.reshape(128, 2 * w)
    else:
        pat = None
    _program_cache[key] = (nc, pat)
    return nc, pat


def kernel(**inputs) -> np.ndarray:
    global LAST_RESULT
    query = np.asarray(inputs["query"], np.float32)
    key = np.asarray(inputs["key"], np.float32)
    value = np.asarray(inputs["value"], np.float32)
    mask = np.asarray(inputs["mask"])
    Wq = np.asarray(inputs["Wq"], np.float32)
    bq = np.asarray(inputs["bq"], np.float32)
    Wk = np.asarray(inputs["Wk"], np.float32)
    bk = np.asarray(inputs["bk"], np.float32)
    Wv = np.asarray(inputs["Wv"], np.float32)
    bv = np.asarray(inputs["bv"], np.float32)
    Wo = np.asarray(inputs["Wo"], np.float32)
    bo = np.asarray(inputs["bo"], np.float32)

    nc, pat = _get_program(mask.reshape(S, S))

    WqT, WkT, WvT, WoT = Wq.T, Wk.T, Wv.T, Wo.T
    xT = {
        t: [np.ascontiguousarray(x[b].T).astype(BFNP) for b in range(B)]
        for t, x in (("qT", query), ("kT", key), ("vT", value))
    }
    in_maps = []
    for c in range(NCORE):
        b, g = divmod(c, GROUPS)
        f0 = g * FPC
        m = {
            "qT": xT["qT"][b],
            "kT": xT["kT"][b],
            "vT": xT["vT"][b],
            "wqT": np.ascontiguousarray(WqT[:, f0:f0 + FPC]).astype(BFNP),
            "wkT": np.ascontiguousarray(WkT[:, f0:f0 + FPC]).astype(BFNP),
            "wvT": np.ascontiguousarray(WvT[:, f0:f0 + FPC]).astype(BFNP),
            "woT": np.ascontiguousarray(WoT[f0:f0 + FPC, :]).astype(BFNP),
            "bqk": np.stack([bq[f0:f0 + FPC].reshape(FT, 128),
                             bk[f0:f0 + FPC].reshape(FT, 128)]).astype(np.float32),
        }
        if pat is not None:
            m["masks"] = pat
        in_maps.append(m)

    res = run_bass_kernel_spmd(
        nc, in_maps, core_ids=list(range(NCORE)),
        trace=PROFILE,
        trace_cores=(TRACE_CORES if TRACE_CORES is not None
                     else (list(range(NCORE)) if PROFILE else None)),
    )
    LAST_RESULT = res

    host_bias = bo + bv @ WoT  # (D,) folded V/O biases, added once per batch
    out = np.empty((B, S, D), np.float32)
    for b in range(B):
        acc = res.results[b * GROUPS]["out"].astype(np.float32)
        for g in range(1, GROUPS):
            acc = acc + res.results[b * GROUPS + g]["out"].astype(np.float32)
        out[b] = acc + host_bias
    return out


# revision 51
# speedup vs baseline: 1.4723x; 1.0021x over previous
"""Multi-head attention (B=2, S=2048, D=1024, H=16) on 8 Trainium2 NeuronCores.

Sharding: core c -> (batch b = c//4, head-group g = c%4).  Each core computes
Q/K/V projections for its 4 heads (256 features), causal attention for those
heads over the full sequence, and a partial O-projection (its 256 attn
features x full Wo.T slice).  The host sums the 4 bf16 partial outputs per
batch and folds in the biases that commute with the reduction (bo, bv @ Wo.T).

Pipeline (per core): the sequence is processed in 4 chunks of 512 tokens;
attention for query block c needs only K/V chunks 0..c, so the serial
ScalarE exp stream starts as soon as chunk 0 is projected.  All eight
(query-block, head-pair) attention blocks run as ONE merged k-tile stream
whose PV matmuls lag one step behind QK/exp (across block boundaries), and
projection / O-projection matmul groups are paced into the stream's slack
from an explicit task queue with per-step deadlines.

Engine assignment:
  TensorE   projections + QK^T + PV + O-proj.  QK^T contracts over dk=64, so
            the two heads of a feature tile (partitions 0-63 / 64-127) issue
            back-to-back and run concurrently in different PE row groups
            (tile_position auto-derived from base partitions).
  ScalarE   only exp (fused 1/sqrt(dk) scale), one ACTIVATE per k-tile
            covering both heads of the pair via a [128, 2, QB] PSUM tile.
  VectorE   PSUM evacuations (+ K/Q bias add), mask multiplies, reciprocal,
            normalize multiplies.
  GpSimd    partition-broadcast of 1/denom.
  V layout  [tok, feat+1] with a ones column: PV also accumulates the
            softmax denominator (scores are O(5): no max subtraction needed,
            f32 exp cannot overflow).
"""

import hashlib
from contextlib import ExitStack

import ml_dtypes
import numpy as np

import concourse.bass as bass
import concourse.tile as tile
from concourse import bacc, mybir
from concourse.bass_utils import run_bass_kernel_spmd

B, S, D, H = 2, 2048, 1024, 16
DK = D // H                  # 64 head dim
NCORE = 8
GROUPS = NCORE // B          # 4 head-groups per batch
HPC = H // GROUPS            # 4 heads per core
FPC = HPC * DK               # 256 features per core
FT = FPC // 128              # 2 feature tiles (= head pairs) per core
DT = D // 128                # 8 d_in tiles
TT = S // 128                # 16 token tiles (k tiles)
QB = 512                     # query block (free-dim) size in attention
NQB = S // QB                # 4 query blocks = pipeline stages
BF = mybir.dt.bfloat16
F32 = mybir.dt.float32
BFNP = ml_dtypes.bfloat16

# module-level knobs for test.py
PROFILE = False
TRACE_CORES = None
LAST_RESULT = None

_program_cache: dict = {}


def _classify_mask(mask2d: np.ndarray):
    """Classify (S, S) keep-mask into per-(qblock, ktile) modes.

    Returns (plan, patterns): plan[qb] is a list of (kt, mask_id|None, c0, c1)
    for tiles that are at least partially kept, where c0 is the first
    q-column (within the block) with any kept key and [c0, c1) the columns
    needing the multiplicative mask; patterns[mid] is a [128, 2, c1-c0] bf16
    multiplicative mask (k on partitions, duplicated across the head axis).
    """
    keep = np.asarray(mask2d) != 0
    patterns = []
    pattern_ids = {}
    plan = []
    for qb in range(NQB):
        row = []
        for kt in range(TT):
            blk = keep[qb * QB:(qb + 1) * QB, kt * 128:(kt + 1) * 128].T
            if not blk.any():
                continue
            if blk.all():
                row.append((kt, None, 0, 0))
                continue
            anyk = blk.any(axis=0)
            allk = blk.all(axis=0)
            c0 = int(np.flatnonzero(anyk)[0])
            notall = np.flatnonzero(~allk)
            c1 = int(notall[-1]) + 1 if notall.size else c0
            pat = blk[:, c0:c1]
            key = pat.tobytes()
            mid = pattern_ids.get(key)
            if mid is None:
                mid = len(patterns)
                pattern_ids[key] = mid
                # duplicate across the 2-head axis of the pT tile
                patterns.append(np.repeat(pat[:, None, :], 2, axis=1)
                                .astype(BFNP))
            row.append((kt, mid, c0, c1))
        plan.append(row)
    return plan, patterns


def build_program(plan, npat, pw):
    nc = bacc.Bacc("TRN2", target_bir_lowering=False, debug=False,
                   num_devices=NCORE)
    qT = nc.dram_tensor("qT", (D, S), BF, kind="ExternalInput").ap()
    kT = nc.dram_tensor("kT", (D, S), BF, kind="ExternalInput").ap()
    vT = nc.dram_tensor("vT", (D, S), BF, kind="ExternalInput").ap()
    wqT = nc.dram_tensor("wqT", (D, FPC), BF, kind="ExternalInput").ap()
    wkT = nc.dram_tensor("wkT", (D, FPC), BF, kind="ExternalInput").ap()
    wvT = nc.dram_tensor("wvT", (D, FPC), BF, kind="ExternalInput").ap()
    woT = nc.dram_tensor("woT", (FPC, D), BF, kind="ExternalInput").ap()
    bqk = nc.dram_tensor("bqk", (2, FT, 128), F32, kind="ExternalInput").ap()
    masks = None
    if npat:
        masks = nc.dram_tensor("masks", (npat, 128, 2 * pw), BF,
                               kind="ExternalInput").ap()
    out = nc.dram_tensor("out", (S, D), BF, kind="ExternalOutput").ap()

    with tile.TileContext(nc) as tc, ExitStack() as ctx:
        singles = ctx.enter_context(tc.tile_pool(name="singles", bufs=1))
        xpool = ctx.enter_context(tc.tile_pool(name="xpool", bufs=2))
        ppool = ctx.enter_context(tc.tile_pool(name="ppool", bufs=6))
        npool = ctx.enter_context(tc.tile_pool(name="npool", bufs=2))
        upool = ctx.enter_context(tc.tile_pool(name="upool", bufs=2))
        opool = ctx.enter_context(tc.tile_pool(name="opool", bufs=2))
        psS = ctx.enter_context(tc.tile_pool(name="psS", bufs=2, space="PSUM"))
        psPV = ctx.enter_context(tc.tile_pool(name="psPV", bufs=1, space="PSUM"))

        # ---- SBUF residents ----
        wq_sb = singles.tile([128, DT, FPC], BF)
        wk_sb = singles.tile([128, DT, FPC], BF)
        wv_sb = singles.tile([128, DT, FPC], BF)
        wo_sb = singles.tile([128, FT, D], BF)
        bias_sb = singles.tile([128, 2, FT], F32)
        mask_sb = None
        if npat:
            mask_sb = singles.tile([128, npat, 2 * pw], BF, name="mask_sb")
        q_sb = singles.tile([128, FT, S], BF)
        k_sb = singles.tile([128, FT, S], BF)
        attn_sb = singles.tile([128, FT, S], BF)
        v_sb = singles.tile([128, TT, HPC, DK + 1], BF)

        # ---- input DMA issue order (sync queue, FIFO = priority): the
        # first compute (K0/Q0 projection, then the exp stream) needs
        # wk+xk0+wq+xq0, so those go first ----
        xk_c = [xpool.tile([128, DT, QB], BF, name=f"xk{c}", tag="xk")
                for c in range(NQB)]
        xq_c = [xpool.tile([128, DT, QB], BF, name=f"xq{c}", tag="xq")
                for c in range(NQB)]
        xv_c = [xpool.tile([128, DT, QB], BF, name=f"xv{c}", tag="xv")
                for c in range(NQB)]

        def chunk_dma(x_sb, x_dram, c):
            nc.sync.dma_start(
                x_sb,
                x_dram[:, c * QB:(c + 1) * QB]
                .rearrange("(t p) f -> p t f", p=128))

        # issue order on the sync ring = first-need order: K0/Q0 projection
        # inputs first (they gate the exp stream), then V0, then the rest
        nc.sync.dma_start(bias_sb, bqk.rearrange("a b p -> p a b"))
        nc.sync.dma_start(wk_sb, wkT.rearrange("(t p) f -> p t f", p=128))
        chunk_dma(xk_c[0], kT, 0)
        nc.sync.dma_start(wq_sb, wqT.rearrange("(t p) f -> p t f", p=128))
        chunk_dma(xq_c[0], qT, 0)
        nc.sync.dma_start(wv_sb, wvT.rearrange("(t p) f -> p t f", p=128))
        chunk_dma(xv_c[0], vT, 0)
        if npat:
            nc.sync.dma_start(mask_sb, masks.rearrange("m p f -> p m f"))
        nc.sync.dma_start(wo_sb, woT.rearrange("(t p) f -> p t f", p=128))
        for c in range(1, NQB):
            chunk_dma(xk_c[c], kT, c)
            chunk_dma(xq_c[c], qT, c)
            chunk_dma(xv_c[c], vT, c)

        # trailing ones column of V: PV's last output partition (64 — still
        # 32-aligned for VectorE access) accumulates the softmax denominator
        for tt in range(TT):
            nc.vector.memset(v_sb[:, tt, :, DK:DK + 1], 1.0)

        # PE warm-up: ~6us of dummy matmuls (one accumulation group, so only
        # one PSUM ring slot is held) while the first inputs stream in, so the
        # HAM clock gate is at 8/8 when the real work starts (and doesn't
        # re-throttle before the first projection's inputs land).
        warm_sb = singles.tile([128, QB], BF)
        nc.vector.memset(warm_sb, 0.0)
        ps = psS.tile([128, QB], F32, tag="acc", name="warm_ps")
        for r in range(14):
            nc.tensor.matmul(ps, lhsT=warm_sb[:, 0:128], rhs=warm_sb,
                             start=(r == 0), stop=(r == 13))

        inv_sqrt_dk = float(1.0 / np.sqrt(DK))

        # ---- PE filler tasks: one PSUM accumulation group each, emitted
        # inside the attention loops to fill the PE while ScalarE streams
        # exp.  Cost estimates (ns) pace the interleave. ----

        def kq_task(c, ft, bi):
            # one K^T (bi=1) / Q^T (bi=0) projection group, split into two
            # 4-dt parts so pacing can interleave at sub-group granularity
            x_sb = xq_c[c] if bi == 0 else xk_c[c]
            w_sb = wq_sb if bi == 0 else wk_sb
            y_sb = q_sb if bi == 0 else k_sb
            box = {}

            def part1():
                box["ps"] = psS.tile([128, QB], F32, tag="acc", name="ps")
                for dt in range(4):
                    nc.tensor.matmul(
                        box["ps"],
                        lhsT=w_sb[:, dt, ft * 128:(ft + 1) * 128],
                        rhs=x_sb[:, dt, :],
                        start=(dt == 0), stop=False)

            def part2():
                for dt in range(4, DT):
                    nc.tensor.matmul(
                        box["ps"],
                        lhsT=w_sb[:, dt, ft * 128:(ft + 1) * 128],
                        rhs=x_sb[:, dt, :],
                        start=False, stop=(dt == DT - 1))
                nc.vector.tensor_scalar_add(
                    y_sb[:, ft, c * QB:(c + 1) * QB], box["ps"],
                    bias_sb[:, bi, ft:ft + 1])
            return [(950, part1), (950, part2)]

        def v_task(c, tt):
            # one V projection token-tile group, split into two 4-dt parts
            box = {}

            def part1():
                box["ps"] = psS.tile([128, FPC], F32, tag="acc", name="ps")
                for dt in range(4):
                    nc.tensor.matmul(
                        box["ps"],
                        lhsT=xv_c[c][:, dt, (tt - 4 * c) * 128:
                                     (tt - 4 * c + 1) * 128],
                        rhs=wv_sb[:, dt, :],
                        start=(dt == 0), stop=False)

            def part2():
                for dt in range(4, DT):
                    nc.tensor.matmul(
                        box["ps"],
                        lhsT=xv_c[c][:, dt, (tt - 4 * c) * 128:
                                     (tt - 4 * c + 1) * 128],
                        rhs=wv_sb[:, dt, :],
                        start=False, stop=(dt == DT - 1))
                nc.vector.tensor_copy(
                    v_sb[:, tt, :, 0:DK],
                    box["ps"].rearrange("p (h d) -> p h d", h=HPC))
            return [(550, part1), (550, part2)]

        def o_tasks(qb, tail=False):
            # partial O-projection for query block qb as 8 tasks (one per
            # (token-tile, column-half) PSUM group); each odd token tile
            # ends with its half-block output DMA.  Tail tasks evacuate on
            # the (then idle) ScalarE instead of the busy VectorE.
            obs = {}

            def mk(qt, nch):
                def emit():
                    if qt % 2 == 0 and nch == 0:
                        obs[qt // 2] = opool.tile([128, 2, D], BF, name="ob")
                    ob = obs[qt // 2]
                    ps = psS.tile([128, QB], F32, tag="acc", name="ps")
                    for hd in range(FT):
                        nc.tensor.matmul(
                            ps,
                            lhsT=attn_sb[:, hd,
                                         qb * QB + qt * 128:
                                         qb * QB + (qt + 1) * 128],
                            rhs=wo_sb[:, hd, nch * QB:(nch + 1) * QB],
                            start=(hd == 0), stop=(hd == FT - 1))
                    dst = ob[:, qt % 2, nch * QB:(nch + 1) * QB]
                    if tail:
                        nc.scalar.copy(dst, ps)
                    else:
                        nc.vector.tensor_copy(dst, ps)
                    if qt % 2 == 1 and nch == D // QB - 1:
                        nc.sync.dma_start(
                            out[qb * QB + (qt - 1) * 128:
                                qb * QB + (qt + 1) * 128, :]
                            .rearrange("(t p) f -> p t f", p=128),
                            ob)
                return (520, emit)
            return [mk(qt, nch) for qt in range(QB // 128)
                    for nch in range(D // QB)]

        tasks = []  # FIFO of [cost_ns, emit_fn, key]

        def drain_tasks(*keys):
            """Force-emit queued tasks (all, or those matching keys)."""
            rest = []
            for t in tasks:
                if not keys or t[2] in keys:
                    t[1]()
                else:
                    rest.append(t)
            tasks[:] = rest

        def qk_exp(qb, ft, kt, mid, c0, c1):
            """QK^T (both heads, concurrent row groups) + exp + mask."""
            s = psS.tile([128, 2, QB], F32, tag="qk", name="s")
            for j2, pr in ((0, 0), (1, 64)):
                nc.tensor.matmul(
                    s[:, j2, c0:],
                    lhsT=k_sb[pr:pr + DK, ft, kt * 128:(kt + 1) * 128],
                    rhs=q_sb[pr:pr + DK, ft, qb * QB + c0:(qb + 1) * QB],
                    start=True, stop=True)
            pT = ppool.tile([128, 2, QB], BF, tag="pt", name="pT")
            nc.scalar.activation(pT[:, :, c0:], s[:, :, c0:],
                                 mybir.ActivationFunctionType.Exp,
                                 scale=inv_sqrt_dk)
            if mid is not None and c1 > c0:
                assert mask_sb is not None
                w = c1 - c0
                nc.vector.tensor_mul(
                    pT[:, :, c0:c1], pT[:, :, c0:c1],
                    mask_sb[:, mid, 0:2 * w].rearrange(
                        "p (j w) -> p j w", j=2))
            return pT

        def normalize(qb, ft, pv):
            """attn^T[d, q] = attnU^T[d, q] / denom[q], per column half."""
            HB2 = QB // 2
            for lo in (0, HB2):
                den = npool.tile([1, 2, HB2], F32, tag="den", name="den")
                nc.scalar.copy(den, pv[DK:DK + 1, :, lo:lo + HB2])
                u = upool.tile([DK, 2, HB2], BF, tag="u", name="u")
                nc.vector.tensor_copy(u, pv[0:DK, :, lo:lo + HB2])
                rec = npool.tile([1, 2, HB2], F32, tag="rec", name="rec")
                nc.vector.reciprocal_approx_fast(rec, den)
                bc = npool.tile([DK, 2, HB2], F32, tag="bc", name="bc")
                nc.gpsimd.partition_broadcast(bc, rec)
                qcols = slice(qb * QB + lo, qb * QB + lo + HB2)
                nc.vector.tensor_mul(attn_sb[0:DK, ft, qcols],
                                     u[:, 0, :], bc[:, 0, :])
                nc.vector.tensor_mul(attn_sb[64:64 + DK, ft, qcols],
                                     u[:, 1, :], bc[:, 1, :])

        # ---- merged global pipeline over all (block, k-tile) steps ----
        # The PV stream lags the QK/exp stream by one step ACROSS block
        # boundaries, so the serial ScalarE exp stream never stalls at a
        # pair boundary.  Q/K/V projection groups carry global-step
        # deadlines (Q before its block; K/V just before the k-tiles that
        # need them); O-projections pace into whatever slack remains.
        def enq(key, ts):
            tasks.extend([cost, emit, key] for cost, emit in ts)

        blocks = [(qb, ft) for qb in range(NQB) for ft in range(FT)]
        sched = []
        starts = {}
        for qb, ft in blocks:
            starts[(qb, ft)] = len(sched)
            for i2, rec in enumerate(plan[qb]):
                sched.append((qb, ft, i2, rec, len(plan[qb])))
        nsteps = len(sched)

        from collections import defaultdict
        due_map = defaultdict(list)
        enq_map = defaultdict(list)
        for i2 in range(4):
            for p, t in enumerate(v_task(0, i2)):
                due_map[i2 + p].append(t)
        for bi in (0, 1):
            for p, t in enumerate(kq_task(0, 1, bi)):
                due_map[2 + p].append(t)
        for c in range(1, NQB):
            for ft in range(FT):
                g0 = starts[(c, ft)]
                for p, t in enumerate(kq_task(c, ft, 0)):
                    due_map[max(0, g0 - 4 + p)].append(t)
                for p, t in enumerate(kq_task(c, ft, 1)):
                    due_map[g0 + 4 * c - 3 + p].append(t)
            g0 = starts[(c, 0)]
            for j2 in range(4):
                for p, t in enumerate(v_task(c, 4 * c + j2)):
                    due_map[g0 + 4 * c + j2 - 2 + p].append(t)
            # o(c-1) becomes available once pair (c-1, 1) normalizes,
            # which happens at step starts[(c, 0)] + 1
            enq_map[starts[(c, 0)] + 2].append((f"o{c - 1}",
                                                o_tasks(c - 1)))

        for _, part in kq_task(0, 0, 1) + kq_task(0, 0, 0):
            part()

        pv = None
        prev = None
        credit, debt = 0.0, 0.0
        for g in range(nsteps):
            qb, ft, i2, (kt, mid, c0, c1), n = sched[g]
            if i2 == 0:
                pv = psPV.tile([DK + 1, 2, QB], F32, name="pv")
            pT = qk_exp(qb, ft, kt, mid, c0, c1)
            debt += (QB - c0) / 2.4 + 90
            credit += (2 * (QB - c0) + 520) / 1.2
            for t in due_map.get(g, ()):
                t[1]()
                debt += t[0]
            for key, ts in enq_map.get(g, ()):
                enq(key, ts)
            # forget deficits older than a few microseconds: pacing should
            # track the CURRENT slack, not the startup's PE-bound backlog
            debt = max(debt, credit - 4000)
            credit = max(credit, debt - 4000)
            while tasks and debt + tasks[0][0] <= credit + 500:
                t = tasks.pop(0)
                t[1]()
                debt += t[0]
            if prev is not None:
                pqb, pft, pi, pkt, pc0, pn, ppT, ppv = prev
                for j2, h in ((0, 2 * pft), (1, 2 * pft + 1)):
                    nc.tensor.matmul(
                        ppv[:, j2, pc0:], lhsT=v_sb[:, pkt, h, :],
                        rhs=ppT[:, j2, pc0:],
                        start=(pi == 0), stop=(pi == pn - 1))
                debt += 2 * ((QB - pc0) / 2.4 + 50)
                if pi == pn - 1:
                    normalize(pqb, pft, ppv)
            prev = (qb, ft, i2, kt, c0, n, pT, pv)
        pqb, pft, pi, pkt, pc0, pn, ppT, ppv = prev
        for j2, h in ((0, 2 * pft), (1, 2 * pft + 1)):
            nc.tensor.matmul(
                ppv[:, j2, pc0:], lhsT=v_sb[:, pkt, h, :],
                rhs=ppT[:, j2, pc0:],
                start=(pi == 0), stop=(pi == pn - 1))
        normalize(pqb, pft, ppv)
        drain_tasks()
        for cost, emit in o_tasks(NQB - 1, tail=True):
            emit()

    nc.compile()
    return nc


def _get_program(mask2d: np.ndarray):
    key = hashlib.sha1(np.ascontiguousarray(mask2d).tobytes()).hexdigest()
    hit = _program_cache.get(key)
    if hit is not None:
        return hit
    plan, patterns = _classify_mask(mask2d)
    pw = max((p.shape[2] for p in patterns), default=0)
    nc = build_program(plan, len(patterns), pw)
    if patterns:
        pat = np.zeros((len(patterns), 128, 2 * pw), BFNP)
        for i, p in enumerate(patterns):
            w = p.shape[2]
            pat[i, :, :2 * w] = p.reshape(128, 2 * w)
    else:
        pat = None
    _program_cache[key] = (nc, pat)
    return nc, pat


def kernel(**inputs) -> np.ndarray:
    global LAST_RESULT
    query = np.asarray(inputs["query"], np.float32)
    key = np.asarray(inputs["key"], np.float32)
    value = np.asarray(inputs["value"], np.float32)
    mask = np.asarray(inputs["mask"])
    Wq = np.asarray(inputs["Wq"], np.float32)
    bq = np.asarray(inputs["bq"], np.float32)
    Wk = np.asarray(inputs["Wk"], np.float32)
    bk = np.asarray(inputs["bk"], np.float32)
    Wv = np.asarray(inputs["Wv"], np.float32)
    bv = np.asarray(inputs["bv"], np.float32)
    Wo = np.asarray(inputs["Wo"], np.float32)
    bo = np.asarray(inputs["bo"], np.float32)

    nc, pat = _get_program(mask.reshape(S, S))

    WqT, WkT, WvT, WoT = Wq.T, Wk.T, Wv.T, Wo.T
    xT = {
        t: [np.ascontiguousarray(x[b].T).astype(BFNP) for b in range(B)]
        for t, x in (("qT", query), ("kT", key), ("vT", value))
    }
    in_maps = []
    for c in range(NCORE):
        b, g = divmod(c, GROUPS)
        f0 = g * FPC
        m = {
            "qT": xT["qT"][b],
            "kT": xT["kT"][b],
            "vT": xT["vT"][b],
            "wqT": np.ascontiguousarray(WqT[:, f0:f0 + FPC]).astype(BFNP),
            "wkT": np.ascontiguousarray(WkT[:, f0:f0 + FPC]).astype(BFNP),
            "wvT": np.ascontiguousarray(WvT[:, f0:f0 + FPC]).astype(BFNP),
            "woT": np.ascontiguousarray(WoT[f0:f0 + FPC, :]).astype(BFNP),
            "bqk": np.stack([bq[f0:f0 + FPC].reshape(FT, 128),
                             bk[f0:f0 + FPC].reshape(FT, 128)]).astype(np.float32),
        }
        if pat is not None:
            m["masks"] = pat
        in_maps.append(m)

    res = run_bass_kernel_spmd(
        nc, in_maps, core_ids=list(range(NCORE)),
        trace=PROFILE,
        trace_cores=(TRACE_CORES if TRACE_CORES is not None
                     else (list(range(NCORE)) if PROFILE else None)),
    )
    LAST_RESULT = res

    host_bias = bo + bv @ WoT  # (D,) folded V/O biases, added once per batch
    out = np.empty((B, S, D), np.float32)
    for b in range(B):
        acc = res.results[b * GROUPS]["out"].astype(np.float32)
        for g in range(1, GROUPS):
            acc = acc + res.results[b * GROUPS + g]["out"].astype(np.float32)
        out[b] = acc + host_bias
    return out


# revision 52
# speedup vs baseline: 1.4803x; 1.0054x over previous
"""Multi-head attention (B=2, S=2048, D=1024, H=16) on 8 Trainium2 NeuronCores.

Sharding: core c -> (batch b = c//4, head-group g = c%4).  Each core computes
Q/K/V projections for its 4 heads (256 features), causal attention for those
heads over the full sequence, and a partial O-projection (its 256 attn
features x full Wo.T slice).  The host sums the 4 bf16 partial outputs per
batch and folds in the biases that commute with the reduction (bo, bv @ Wo.T).

Pipeline (per core): the sequence is processed in 4 chunks of 512 tokens;
attention for query block c needs only K/V chunks 0..c, so the serial
ScalarE exp stream starts as soon as chunk 0 is projected.  All eight
(query-block, head-pair) attention blocks run as ONE merged k-tile stream
whose PV matmuls lag one step behind QK/exp (across block boundaries), and
projection / O-projection matmul groups are paced into the stream's slack
from an explicit task queue with per-step deadlines.

Engine assignment:
  TensorE   projections + QK^T + PV + O-proj.  QK^T contracts over dk=64, so
            the two heads of a feature tile (partitions 0-63 / 64-127) issue
            back-to-back and run concurrently in different PE row groups
            (tile_position auto-derived from base partitions).
  ScalarE   only exp (fused 1/sqrt(dk) scale), one ACTIVATE per k-tile
            covering both heads of the pair via a [128, 2, QB] PSUM tile.
  VectorE   PSUM evacuations (+ K/Q bias add), mask multiplies, reciprocal,
            normalize multiplies.
  GpSimd    partition-broadcast of 1/denom.
  V layout  [tok, feat+1] with a ones column: PV also accumulates the
            softmax denominator (scores are O(5): no max subtraction needed,
            f32 exp cannot overflow).
"""

import hashlib
from contextlib import ExitStack

import ml_dtypes
import numpy as np

import concourse.bass as bass
import concourse.tile as tile
from concourse import bacc, mybir
from concourse.bass_utils import run_bass_kernel_spmd

B, S, D, H = 2, 2048, 1024, 16
DK = D // H                  # 64 head dim
NCORE = 8
GROUPS = NCORE // B          # 4 head-groups per batch
HPC = H // GROUPS            # 4 heads per core
FPC = HPC * DK               # 256 features per core
FT = FPC // 128              # 2 feature tiles (= head pairs) per core
DT = D // 128                # 8 d_in tiles
TT = S // 128                # 16 token tiles (k tiles)
QB = 512                     # query block (free-dim) size in attention
NQB = S // QB                # 4 query blocks = pipeline stages
BF = mybir.dt.bfloat16
F32 = mybir.dt.float32
BFNP = ml_dtypes.bfloat16

# module-level knobs for test.py
PROFILE = False
TRACE_CORES = None
LAST_RESULT = None

_program_cache: dict = {}


def _classify_mask(mask2d: np.ndarray):
    """Classify (S, S) keep-mask into per-(qblock, ktile) modes.

    Returns (plan, patterns): plan[qb] is a list of (kt, mask_id|None, c0, c1)
    for tiles that are at least partially kept, where c0 is the first
    q-column (within the block) with any kept key and [c0, c1) the columns
    needing the multiplicative mask; patterns[mid] is a [128, 2, c1-c0] bf16
    multiplicative mask (k on partitions, duplicated across the head axis).
    """
    keep = np.asarray(mask2d) != 0
    patterns = []
    pattern_ids = {}
    plan = []
    for qb in range(NQB):
        row = []
        for kt in range(TT):
            blk = keep[qb * QB:(qb + 1) * QB, kt * 128:(kt + 1) * 128].T
            if not blk.any():
                continue
            if blk.all():
                row.append((kt, None, 0, 0))
                continue
            anyk = blk.any(axis=0)
            allk = blk.all(axis=0)
            c0 = int(np.flatnonzero(anyk)[0])
            notall = np.flatnonzero(~allk)
            c1 = int(notall[-1]) + 1 if notall.size else c0
            pat = blk[:, c0:c1]
            key = pat.tobytes()
            mid = pattern_ids.get(key)
            if mid is None:
                mid = len(patterns)
                pattern_ids[key] = mid
                # duplicate across the 2-head axis of the pT tile
                patterns.append(np.repeat(pat[:, None, :], 2, axis=1)
                                .astype(BFNP))
            row.append((kt, mid, c0, c1))
        plan.append(row)
    return plan, patterns


def build_program(plan, npat, pw):
    nc = bacc.Bacc("TRN2", target_bir_lowering=False, debug=False,
                   num_devices=NCORE)
    qT = nc.dram_tensor("qT", (D, S), BF, kind="ExternalInput").ap()
    kT = nc.dram_tensor("kT", (D, S), BF, kind="ExternalInput").ap()
    vT = nc.dram_tensor("vT", (D, S), BF, kind="ExternalInput").ap()
    wqT = nc.dram_tensor("wqT", (D, FPC), BF, kind="ExternalInput").ap()
    wkT = nc.dram_tensor("wkT", (D, FPC), BF, kind="ExternalInput").ap()
    wvT = nc.dram_tensor("wvT", (D, FPC), BF, kind="ExternalInput").ap()
    woT = nc.dram_tensor("woT", (FPC, D), BF, kind="ExternalInput").ap()
    bqk = nc.dram_tensor("bqk", (2, FT, 128), F32, kind="ExternalInput").ap()
    masks = None
    if npat:
        masks = nc.dram_tensor("masks", (npat, 128, 2 * pw), BF,
                               kind="ExternalInput").ap()
    out = nc.dram_tensor("out", (S, D), BF, kind="ExternalOutput").ap()

    with tile.TileContext(nc) as tc, ExitStack() as ctx:
        singles = ctx.enter_context(tc.tile_pool(name="singles", bufs=1))
        xpool = ctx.enter_context(tc.tile_pool(name="xpool", bufs=2))
        ppool = ctx.enter_context(tc.tile_pool(name="ppool", bufs=6))
        npool = ctx.enter_context(tc.tile_pool(name="npool", bufs=2))
        upool = ctx.enter_context(tc.tile_pool(name="upool", bufs=2))
        opool = ctx.enter_context(tc.tile_pool(name="opool", bufs=2))
        psS = ctx.enter_context(tc.tile_pool(name="psS", bufs=2, space="PSUM"))
        psPV = ctx.enter_context(tc.tile_pool(name="psPV", bufs=1, space="PSUM"))

        # ---- SBUF residents ----
        wq_sb = singles.tile([128, DT, FPC], BF)
        wk_sb = singles.tile([128, DT, FPC], BF)
        wv_sb = singles.tile([128, DT, FPC], BF)
        wo_sb = singles.tile([128, FT, D], BF)
        bias_sb = singles.tile([128, 2, FT], F32)
        mask_sb = None
        if npat:
            mask_sb = singles.tile([128, npat, 2 * pw], BF, name="mask_sb")
        q_sb = singles.tile([128, FT, S], BF)
        k_sb = singles.tile([128, FT, S], BF)
        attn_sb = singles.tile([128, FT, S], BF)
        v_sb = singles.tile([128, TT, HPC, DK + 1], BF)

        # ---- input DMA issue order (sync queue, FIFO = priority): the
        # first compute (K0/Q0 projection, then the exp stream) needs
        # wk+xk0+wq+xq0, so those go first ----
        xk_c = [xpool.tile([128, DT, QB], BF, name=f"xk{c}", tag="xk")
                for c in range(NQB)]
        xq_c = [xpool.tile([128, DT, QB], BF, name=f"xq{c}", tag="xq")
                for c in range(NQB)]
        xv_c = [xpool.tile([128, DT, QB], BF, name=f"xv{c}", tag="xv")
                for c in range(NQB)]

        def chunk_dma(x_sb, x_dram, c):
            nc.sync.dma_start(
                x_sb,
                x_dram[:, c * QB:(c + 1) * QB]
                .rearrange("(t p) f -> p t f", p=128))

        # issue order on the sync ring = first-need order: K0/Q0 projection
        # inputs first (they gate the exp stream), then V0, then the rest
        nc.sync.dma_start(bias_sb, bqk.rearrange("a b p -> p a b"))
        nc.sync.dma_start(wk_sb, wkT.rearrange("(t p) f -> p t f", p=128))
        chunk_dma(xk_c[0], kT, 0)
        nc.sync.dma_start(wq_sb, wqT.rearrange("(t p) f -> p t f", p=128))
        chunk_dma(xq_c[0], qT, 0)
        nc.sync.dma_start(wv_sb, wvT.rearrange("(t p) f -> p t f", p=128))
        chunk_dma(xv_c[0], vT, 0)
        if npat:
            nc.sync.dma_start(mask_sb, masks.rearrange("m p f -> p m f"))
        nc.sync.dma_start(wo_sb, woT.rearrange("(t p) f -> p t f", p=128))
        for c in range(1, NQB):
            chunk_dma(xk_c[c], kT, c)
            chunk_dma(xq_c[c], qT, c)
            chunk_dma(xv_c[c], vT, c)

        # trailing ones column of V: PV's last output partition (64 — still
        # 32-aligned for VectorE access) accumulates the softmax denominator
        for tt in range(TT):
            nc.vector.memset(v_sb[:, tt, :, DK:DK + 1], 1.0)

        # PE warm-up: ~6us of dummy matmuls (one accumulation group, so only
        # one PSUM ring slot is held) while the first inputs stream in, so the
        # HAM clock gate is at 8/8 when the real work starts (and doesn't
        # re-throttle before the first projection's inputs land).
        warm_sb = singles.tile([128, QB], BF)
        nc.vector.memset(warm_sb, 0.0)
        ps = psS.tile([128, QB], F32, tag="acc", name="warm_ps")
        for r in range(14):
            nc.tensor.matmul(ps, lhsT=warm_sb[:, 0:128], rhs=warm_sb,
                             start=(r == 0), stop=(r == 13))

        inv_sqrt_dk = float(1.0 / np.sqrt(DK))

        # ---- PE filler tasks: one PSUM accumulation group each, emitted
        # inside the attention loops to fill the PE while ScalarE streams
        # exp.  Cost estimates (ns) pace the interleave. ----

        def kq_task(c, ft, bi):
            # one K^T (bi=1) / Q^T (bi=0) projection group, split into two
            # 4-dt parts so pacing can interleave at sub-group granularity
            x_sb = xq_c[c] if bi == 0 else xk_c[c]
            w_sb = wq_sb if bi == 0 else wk_sb
            y_sb = q_sb if bi == 0 else k_sb
            box = {}

            def part1():
                box["ps"] = psS.tile([128, QB], F32, tag="acc", name="ps")
                for dt in range(4):
                    nc.tensor.matmul(
                        box["ps"],
                        lhsT=w_sb[:, dt, ft * 128:(ft + 1) * 128],
                        rhs=x_sb[:, dt, :],
                        start=(dt == 0), stop=False)

            def part2():
                for dt in range(4, DT):
                    nc.tensor.matmul(
                        box["ps"],
                        lhsT=w_sb[:, dt, ft * 128:(ft + 1) * 128],
                        rhs=x_sb[:, dt, :],
                        start=False, stop=(dt == DT - 1))
                nc.vector.tensor_scalar_add(
                    y_sb[:, ft, c * QB:(c + 1) * QB], box["ps"],
                    bias_sb[:, bi, ft:ft + 1])
            return [(950, part1), (950, part2)]

        def v_task(c, tt):
            # one V projection token-tile group, split into two 4-dt parts
            box = {}

            def part1():
                box["ps"] = psS.tile([128, FPC], F32, tag="acc", name="ps")
                for dt in range(4):
                    nc.tensor.matmul(
                        box["ps"],
                        lhsT=xv_c[c][:, dt, (tt - 4 * c) * 128:
                                     (tt - 4 * c + 1) * 128],
                        rhs=wv_sb[:, dt, :],
                        start=(dt == 0), stop=False)

            def part2():
                for dt in range(4, DT):
                    nc.tensor.matmul(
                        box["ps"],
                        lhsT=xv_c[c][:, dt, (tt - 4 * c) * 128:
                                     (tt - 4 * c + 1) * 128],
                        rhs=wv_sb[:, dt, :],
                        start=False, stop=(dt == DT - 1))
                nc.vector.tensor_copy(
                    v_sb[:, tt, :, 0:DK],
                    box["ps"].rearrange("p (h d) -> p h d", h=HPC))
            return [(550, part1), (550, part2)]

        def o_tasks(qb, tail=False):
            # partial O-projection for query block qb as 8 tasks (one per
            # (token-tile, column-half) PSUM group); each odd token tile
            # ends with its half-block output DMA.  Tail tasks evacuate on
            # the (then idle) ScalarE instead of the busy VectorE.
            obs = {}

            def mk(qt, nch):
                def emit():
                    if qt % 2 == 0 and nch == 0:
                        obs[qt // 2] = opool.tile([128, 2, D], BF, name="ob")
                    ob = obs[qt // 2]
                    ps = psS.tile([128, QB], F32, tag="acc", name="ps")
                    for hd in range(FT):
                        nc.tensor.matmul(
                            ps,
                            lhsT=attn_sb[:, hd,
                                         qb * QB + qt * 128:
                                         qb * QB + (qt + 1) * 128],
                            rhs=wo_sb[:, hd, nch * QB:(nch + 1) * QB],
                            start=(hd == 0), stop=(hd == FT - 1))
                    dst = ob[:, qt % 2, nch * QB:(nch + 1) * QB]
                    if tail:
                        nc.scalar.copy(dst, ps)
                    else:
                        nc.vector.tensor_copy(dst, ps)
                    if qt % 2 == 1 and nch == D // QB - 1:
                        nc.sync.dma_start(
                            out[qb * QB + (qt - 1) * 128:
                                qb * QB + (qt + 1) * 128, :]
                            .rearrange("(t p) f -> p t f", p=128),
                            ob)
                return (520, emit)
            return [mk(qt, nch) for qt in range(QB // 128)
                    for nch in range(D // QB)]

        tasks = []  # FIFO of [cost_ns, emit_fn, key]

        def drain_tasks(*keys):
            """Force-emit queued tasks (all, or those matching keys)."""
            rest = []
            for t in tasks:
                if not keys or t[2] in keys:
                    t[1]()
                else:
                    rest.append(t)
            tasks[:] = rest

        def qk_exp(qb, ft, kt, mid, c0, c1):
            """QK^T (both heads, concurrent row groups) + exp + mask."""
            s = psS.tile([128, 2, QB], F32, tag="qk", name="s")
            for j2, pr in ((0, 0), (1, 64)):
                nc.tensor.matmul(
                    s[:, j2, c0:],
                    lhsT=k_sb[pr:pr + DK, ft, kt * 128:(kt + 1) * 128],
                    rhs=q_sb[pr:pr + DK, ft, qb * QB + c0:(qb + 1) * QB],
                    start=True, stop=True)
            pT = ppool.tile([128, 2, QB], BF, tag="pt", name="pT")
            nc.scalar.activation(pT[:, :, c0:], s[:, :, c0:],
                                 mybir.ActivationFunctionType.Exp,
                                 scale=inv_sqrt_dk)
            if mid is not None and c1 > c0:
                assert mask_sb is not None
                w = c1 - c0
                nc.vector.tensor_mul(
                    pT[:, :, c0:c1], pT[:, :, c0:c1],
                    mask_sb[:, mid, 0:2 * w].rearrange(
                        "p (j w) -> p j w", j=2))
            return pT

        def normalize(qb, ft, pv):
            """attn^T[d, q] = attnU^T[d, q] / denom[q], per column half."""
            HB2 = QB // 2
            for lo in (0, HB2):
                den = npool.tile([1, 2, HB2], F32, tag="den", name="den")
                nc.scalar.copy(den, pv[DK:DK + 1, :, lo:lo + HB2])
                u = upool.tile([DK, 2, HB2], BF, tag="u", name="u")
                nc.vector.tensor_copy(u, pv[0:DK, :, lo:lo + HB2])
                rec = npool.tile([1, 2, HB2], F32, tag="rec", name="rec")
                nc.vector.reciprocal_approx_fast(rec, den)
                bc = npool.tile([DK, 2, HB2], F32, tag="bc", name="bc")
                nc.gpsimd.partition_broadcast(bc, rec)
                qcols = slice(qb * QB + lo, qb * QB + lo + HB2)
                nc.vector.tensor_mul(attn_sb[0:DK, ft, qcols],
                                     u[:, 0, :], bc[:, 0, :])
                nc.vector.tensor_mul(attn_sb[64:64 + DK, ft, qcols],
                                     u[:, 1, :], bc[:, 1, :])

        # ---- merged global pipeline over all (block, k-tile) steps ----
        # The PV stream lags the QK/exp stream by one step ACROSS block
        # boundaries, so the serial ScalarE exp stream never stalls at a
        # pair boundary.  Q/K/V projection groups carry global-step
        # deadlines (Q before its block; K/V just before the k-tiles that
        # need them); O-projections pace into whatever slack remains.
        def enq(key, ts):
            tasks.extend([cost, emit, key] for cost, emit in ts)

        blocks = [(qb, ft) for qb in range(NQB) for ft in range(FT)]
        sched = []
        starts = {}
        for qb, ft in blocks:
            starts[(qb, ft)] = len(sched)
            for i2, rec in enumerate(plan[qb]):
                sched.append((qb, ft, i2, rec, len(plan[qb])))
        nsteps = len(sched)

        from collections import defaultdict
        due_map = defaultdict(list)
        enq_map = defaultdict(list)
        for i2 in range(4):
            for p, t in enumerate(v_task(0, i2)):
                due_map[i2 + p].append(t)
        for bi in (0, 1):
            for p, t in enumerate(kq_task(0, 1, bi)):
                due_map[2 + p].append(t)
        for c in range(1, NQB):
            for ft in range(FT):
                g0 = starts[(c, ft)]
                for p, t in enumerate(kq_task(c, ft, 0)):
                    due_map[max(0, g0 - 4 + p)].append(t)
                for p, t in enumerate(kq_task(c, ft, 1)):
                    due_map[g0 + 4 * c - 3 + p].append(t)
            g0 = starts[(c, 0)]
            for j2 in range(4):
                for p, t in enumerate(v_task(c, 4 * c + j2)):
                    due_map[g0 + 4 * c + j2 - 2 + p].append(t)
            # o(c-1) becomes available once pair (c-1, 1) normalizes,
            # which happens at step starts[(c, 0)] + 1
            enq_map[starts[(c, 0)] + 2].append((f"o{c - 1}",
                                                o_tasks(c - 1)))

        for _, part in kq_task(0, 0, 1) + kq_task(0, 0, 0):
            part()

        pv = None
        prev = None
        credit, debt = 0.0, 0.0
        for g in range(nsteps):
            qb, ft, i2, (kt, mid, c0, c1), n = sched[g]
            if i2 == 0:
                pv = psPV.tile([DK + 1, 2, QB], F32, name="pv")
            pT = qk_exp(qb, ft, kt, mid, c0, c1)
            debt += (QB - c0) / 2.4 + 90
            credit += (2 * (QB - c0) + 520) / 1.2
            for t in due_map.get(g, ()):
                t[1]()
                debt += t[0]
            for key, ts in enq_map.get(g, ()):
                enq(key, ts)
            # forget deficits older than a few microseconds: pacing should
            # track the CURRENT slack, not the startup's PE-bound backlog
            debt = max(debt, credit - 4000)
            credit = max(credit, debt - 4000)
            # cap filler per step: a burst between two QKs delays the next
            # exp by the burst length, punching a hole in the exp stream
            step_fill = 0
            while (tasks and debt + tasks[0][0] <= credit + 500
                   and step_fill + tasks[0][0] <= 1100):
                t = tasks.pop(0)
                t[1]()
                debt += t[0]
                step_fill += t[0]
            if prev is not None:
                pqb, pft, pi, pkt, pc0, pn, ppT, ppv = prev
                for j2, h in ((0, 2 * pft), (1, 2 * pft + 1)):
                    nc.tensor.matmul(
                        ppv[:, j2, pc0:], lhsT=v_sb[:, pkt, h, :],
                        rhs=ppT[:, j2, pc0:],
                        start=(pi == 0), stop=(pi == pn - 1))
                debt += 2 * ((QB - pc0) / 2.4 + 50)
                if pi == pn - 1:
                    normalize(pqb, pft, ppv)
            prev = (qb, ft, i2, kt, c0, n, pT, pv)
        pqb, pft, pi, pkt, pc0, pn, ppT, ppv = prev
        for j2, h in ((0, 2 * pft), (1, 2 * pft + 1)):
            nc.tensor.matmul(
                ppv[:, j2, pc0:], lhsT=v_sb[:, pkt, h, :],
                rhs=ppT[:, j2, pc0:],
                start=(pi == 0), stop=(pi == pn - 1))
        normalize(pqb, pft, ppv)
        drain_tasks()
        for cost, emit in o_tasks(NQB - 1, tail=True):
            emit()

    nc.compile()
    return nc


def _get_program(mask2d: np.ndarray):
    key = hashlib.sha1(np.ascontiguousarray(mask2d).tobytes()).hexdigest()
    hit = _program_cache.get(key)
    if hit is not None:
        return hit
    plan, patterns = _classify_mask(mask2d)
    pw = max((p.shape[2] for p in patterns), default=0)
    nc = build_program(plan, len(patterns), pw)
    if patterns:
        pat = np.zeros((len(patterns), 128, 2 * pw), BFNP)
        for i, p in enumerate(patterns):
            w = p.shape[2]
            pat[i, :, :2 * w] = p.reshape(128, 2 * w)
    else:
        pat = None
    _program_cache[key] = (nc, pat)
    return nc, pat


def kernel(**inputs) -> np.ndarray:
    global LAST_RESULT
    query = np.asarray(inputs["query"], np.float32)
    key = np.asarray(inputs["key"], np.float32)
    value = np.asarray(inputs["value"], np.float32)
    mask = np.asarray(inputs["mask"])
    Wq = np.asarray(inputs["Wq"], np.float32)
    bq = np.asarray(inputs["bq"], np.float32)
    Wk = np.asarray(inputs["Wk"], np.float32)
    bk = np.asarray(inputs["bk"], np.float32)
    Wv = np.asarray(inputs["Wv"], np.float32)
    bv = np.asarray(inputs["bv"], np.float32)
    Wo = np.asarray(inputs["Wo"], np.float32)
    bo = np.asarray(inputs["bo"], np.float32)

    nc, pat = _get_program(mask.reshape(S, S))

    WqT, WkT, WvT, WoT = Wq.T, Wk.T, Wv.T, Wo.T
    xT = {
        t: [np.ascontiguousarray(x[b].T).astype(BFNP) for b in range(B)]
        for t, x in (("qT", query), ("kT", key), ("vT", value))
    }
    in_maps = []
    for c in range(NCORE):
        b, g = divmod(c, GROUPS)
        f0 = g * FPC
        m = {
            "qT": xT["qT"][b],
            "kT": xT["kT"][b],
            "vT": xT["vT"][b],
            "wqT": np.ascontiguousarray(WqT[:, f0:f0 + FPC]).astype(BFNP),
            "wkT": np.ascontiguousarray(WkT[:, f0:f0 + FPC]).astype(BFNP),
            "wvT": np.ascontiguousarray(WvT[:, f0:f0 + FPC]).astype(BFNP),
            "woT": np.ascontiguousarray(WoT[f0:f0 + FPC, :]).astype(BFNP),
            "bqk": np.stack([bq[f0:f0 + FPC].reshape(FT, 128),
                             bk[f0:f0 + FPC].reshape(FT, 128)]).astype(np.float32),
        }
        if pat is not None:
            m["masks"] = pat
        in_maps.append(m)

    res = run_bass_kernel_spmd(
        nc, in_maps, core_ids=list(range(NCORE)),
        trace=PROFILE,
        trace_cores=(TRACE_CORES if TRACE_CORES is not None
                     else (list(range(NCORE)) if PROFILE else None)),
    )
    LAST_RESULT = res

    host_bias = bo + bv @ WoT  # (D,) folded V/O biases, added once per batch
    out = np.empty((B, S, D), np.float32)
    for b in range(B):
        acc = res.results[b * GROUPS]["out"].astype(np.float32)
        for g in range(1, GROUPS):
            acc = acc + res.results[b * GROUPS + g]["out"].astype(np.float32)
        out[b] = acc + host_bias
    return out
